# revision 11
# baseline (speedup 1.0000x reference)
"""nn_BinaryMoSLinear Trainium2 kernel: 8 NeuronCores, 2 token-halves x 4
out-feature quarters.

kernel(**inputs) takes the FULL reference.setup_inputs() tensors and returns
the FULL [4, 2048, 4096] f32 output. Core c handles token half ti=c//4 and
out-feature quarter oi=c%4 (Nc=4096 tokens, Oc=1024 features per core).

The host pre-binarizes the weight (sign -> bf16, exactly representable) and
casts/transposes x to bf16, so the main matmul runs with a stationary bf16
weight tile [128h x 128o] and a moving activation tile [128h x 512tok] at
1 row/cycle. The token stream is chunked into 8 pairs of 512: while pair k's
256 main matmuls run, pair k+1's router/softmax/in-scale/xs stages are
interleaved into the 8 o-blocks (in-scale matmuls woven into the main
h-loop) so the PE never starves. Output is produced transposed [Oc, Nc]
and re-assembled on the host.
"""
from contextlib import ExitStack

import concourse.bass as bass
import concourse.mybir as mybir

F32 = mybir.dt.float32
F32R = mybir.dt.float32r
BF16 = mybir.dt.bfloat16
AF = mybir.ActivationFunctionType
OP = mybir.AluOpType


def f32(ap):
    return ap.bitcast(F32)


def build_kernel(ctx, tc, outs, ins, cfg):
    nc = tc.nc
    H, Oc, Nc, E = cfg["H"], cfg["Oc"], cfg["Nc"], cfg["E"]
    NH = H // 128            # 32 h-chunks
    NO = Oc // 128           # 8 o-tiles
    NP = Nc // 512           # 8 token pairs (tc chunks)
    RING = 2
    y = outs["y"]

    pool = ctx.enter_context(tc.tile_pool(name="sb", bufs=1))

    # ---- persistent SBUF tensors ----
    ring = [pool.tile([128, NH * 512], BF16, name=f"ring{r}", tag=f"ring{r}",
                      bufs=1) for r in range(RING)]
    wq = [pool.tile([128, NH * 256], BF16, name=f"wq{q}", tag=f"wq{q}",
                    bufs=1) for q in range(4)]
    rT_rep = pool.tile([128, Nc], F32R, name="rT_rep", tag="rT", bufs=1)
    ics_rep = pool.tile([128, H], F32R, name="ics_rep", tag="ics", bufs=1)
    gw_pack = pool.tile([128, NH * E], BF16, name="gw_pack", tag="gw", bufs=1)
    ocs_sb = pool.tile([E, Oc], F32R, name="ocs_sb", tag="ocs", bufs=1)
    bias_sb = pool.tile([128, NO], F32, name="bias_sb", tag="bias", bufs=1)
    sel_sb = pool.tile([128, E], F32R, name="sel_sb", tag="sel", bufs=1)
    ones_e1 = pool.tile([E, 1], F32R, name="ones_e1", tag="o41", bufs=1)
    ones_1e = pool.tile([1, E], F32R, name="ones_1e", tag="o14", bufs=1)

    # ---- constant DMAs ----
    nc.sync.dma_start(gw_pack[:], ins["gw"][:, :])
    nc.sync.dma_start(ics_rep[:], ins["icsr"][:, :])
    nc.sync.dma_start(ocs_sb[:], ins["ocs"][:, :])
    nc.sync.dma_start(bias_sb[:], ins["bias"][:, :])
    nc.sync.dma_start(sel_sb[:], ins["sel"][:, :])
    nc.sync.dma_start(ones_e1[:], ins["ones41"][:, :])
    nc.sync.dma_start(ones_1e[:], ins["ones14"][:, :])
    # weight quarters on the ACT hwdge ring (parallel with x on sync ring)
    for q in range(4):
        nc.scalar.dma_start(wq[q][:], ins["wp"][q * 128:(q + 1) * 128, :])

    def x_dma(pr):
        nc.sync.dma_start(ring[pr % RING][:],
                          ins["xp"][pr * 128:(pr + 1) * 128, :])

    # ---- PSUM pools ----
    ps_main = ctx.enter_context(
        tc.tile_pool(name="ps_main", bufs=2, space="PSUM"))
    ps_os = ctx.enter_context(tc.tile_pool(name="ps_os", bufs=2, space="PSUM"))
    ps_sm = ctx.enter_context(tc.tile_pool(name="ps_sm", bufs=2, space="PSUM"))
    ps_is = ctx.enter_context(tc.tile_pool(name="ps_is", bufs=2, space="PSUM"))

    class Prep:
        """Stages preparing pair pr: router -> softmax -> in-scale/xs.

        r (softmax weights) is produced on partition strips {0-3} and
        {32-35} so the K=4 in-scale matmuls can run 2-row-group packed
        without any cross-partition replication step.
        """

        def __init__(self, pr):
            self.pr = pr
            self.rng = ring[pr % RING]
            self.tsl = slice(pr * 512, (pr + 1) * 512)

        def router(self):
            pr = self.pr
            self.strip = ps_sm.tile([128, 512], F32, name=f"strip{pr}",
                                    tag="psm")
            for r in range(NH // 4):
                for c in range(4):
                    h = 4 * r + c
                    nc.tensor.matmul(
                        self.strip[32 * c:32 * c + E, :],
                        gw_pack[:, h * E:(h + 1) * E],
                        self.rng[:, h * 512:(h + 1) * 512],
                        start=(r == 0), stop=(r == NH // 4 - 1),
                        tile_position=(0, 32 * c))

        def sm_a(self):
            pr = self.pr
            sc = pool.tile([128, 512], F32R, name=f"sc{pr}", tag="sc", bufs=2)
            nc.vector.tensor_copy(sc[:], self.strip[:])
            lg = ps_sm.tile([E, 512], F32, name=f"lg{pr}", tag="psm")
            nc.tensor.matmul(lg[:], sel_sb[:], sc[:], start=True, stop=True)
            self.ex = pool.tile([E, 512], F32R, name=f"ex{pr}", tag="ex",
                                bufs=2)
            nc.scalar.activation(self.ex[:], lg[:], AF.Exp)

        def sm_b(self):
            pr = self.pr
            ssum = ps_sm.tile([1, 512], F32, name=f"ss{pr}", tag="psm")
            nc.tensor.matmul(ssum[:], ones_e1[:], self.ex[:],
                             start=True, stop=True)
            self.rcp = pool.tile([1, 512], F32R, name=f"rcp{pr}", tag="rcp",
                                 bufs=2)
            with nc.allow_low_precision(reason="softmax reciprocal"):
                nc.vector.reciprocal(self.rcp[:], ssum[:])

        def sm_c(self):
            pr = self.pr
            bc = ps_sm.tile([E, 512], F32, name=f"bc{pr}", tag="psm")
            nc.tensor.matmul(bc[:], ones_1e[:], self.rcp[:], start=True,
                             stop=True)
            nc.vector.tensor_tensor(rT_rep[0:E, self.tsl], f32(self.ex[:]),
                                    bc[:], OP.mult)
            # replicate r onto partition strip 32-35 for the packed
            # in-scale matmuls (f32r matmuls cannot col-tile to strip 1)
            nc.scalar.dma_start(rT_rep[32:32 + E, self.tsl],
                                rT_rep[0:E, self.tsl])

        def is_round(self, r):
            pr = self.pr
            for c in range(2):
                h = 2 * r + c
                isp = ps_is.tile([128, 512], F32, name=f"is{pr}_{h}",
                                 tag="psi")
                nc.tensor.matmul(
                    isp[:],
                    ics_rep[32 * c:32 * c + E, h * 128:(h + 1) * 128],
                    rT_rep[32 * c:32 * c + E, self.tsl],
                    start=True, stop=True)
                xsl = self.rng[:, h * 512:(h + 1) * 512]
                nc.vector.tensor_tensor(xsl, xsl, isp[:], OP.mult)

        # stage placement inside the consuming TC's o-blocks
        def post(self, o):
            if o == 2:
                self.router()
            elif o == 3:
                self.sm_a()
            elif o == 4:
                self.sm_b()

        def insert(self, o, j):
            if o == 5:
                if j == 3:
                    self.sm_c()
                elif j >= 19 and j % 4 == 3:
                    self.is_round((j - 19) // 4)          # rounds 0..3
            elif o in (6, 7) and j % 4 == 3 and j <= 23:
                self.is_round(4 + 6 * (o - 6) + (j - 3) // 4)  # rounds 4..15

    # ---- prologue: x for pairs 0..1, prep pair 0 inline ----
    for pr in range(min(RING, NP)):
        x_dma(pr)
    prep0 = Prep(0)
    prep0.router()
    prep0.sm_a()
    prep0.sm_b()
    prep0.sm_c()
    for r in range(NH // 2):
        prep0.is_round(r)

    # ---- main loop over token pairs ----
    for k in range(NP):
        prep = Prep(k + 1) if k + 1 < NP else None
        rng = ring[k % RING]
        tsl = slice(k * 512, (k + 1) * 512)
        for o in range(NO):
            osp = ps_os.tile([128, 512], F32, name=f"os{k}_{o}", tag="pso")
            nc.tensor.matmul(osp[:], ocs_sb[:, o * 128:(o + 1) * 128],
                             rT_rep[0:E, tsl], start=True, stop=True)
            os_sb = pool.tile([128, 512], F32, name=f"osb{k}_{o}",
                              tag="osb", bufs=2)
            nc.scalar.activation(os_sb[:], osp[:], AF.Copy)
            mp = ps_main.tile([128, 512], F32, name=f"mp{k}_{o}", tag="psm")
            q, half = o // 2, (o % 2) * 128
            for j in range(NH):
                nc.tensor.matmul(
                    mp[:],
                    wq[q][:, j * 256 + half:j * 256 + half + 128],
                    rng[:, j * 512:(j + 1) * 512],
                    start=(j == 0), stop=(j == NH - 1))
                if prep is not None:
                    prep.insert(o, j)
            if prep is not None:
                prep.post(o)
            y1 = pool.tile([128, 512], F32, name=f"y1_{k}_{o}",
                           tag="y1", bufs=2)
            nc.vector.tensor_tensor(y1[:], mp[:], os_sb[:], OP.mult)
            y2 = pool.tile([128, 512], F32, name=f"y2_{k}_{o}",
                           tag="y2", bufs=3)
            nc.scalar.activation(y2[:], y1[:], AF.Identity,
                                 bias=bias_sb[:, o:o + 1])
            nc.scalar.dma_start(y[o * 128:(o + 1) * 128, tsl], y2[:])
        # refill this ring slot for pair k+RING; emitted after the last
        # reader of the old contents so the overwrite orders correctly
        if k + RING < NP:
            x_dma(k + RING)


import numpy as np
import ml_dtypes

BF = ml_dtypes.bfloat16
NCORES = 8
A, B = 2, 4
Bsz, S, H, O, E = 4, 2048, 4096, 4096, 4
N = Bsz * S
Nc = N // A
Oc = O // B
NH = H // 128
NO = Oc // 128
NP = Nc // 512
CFG = dict(H=H, Oc=Oc, Nc=Nc, E=E)

TRACE = False
LAST_EXEC_NS = None
LAST_TRACE_PATH = None
_NC_CACHE = None


def _get_nc():
    global _NC_CACHE
    if _NC_CACHE is None:
        import concourse.bacc as bacc
        import concourse.tile as tile
        nc = bacc.Bacc("TRN2", target_bir_lowering=False, debug=False,
                       num_devices=NCORES)
        ins_aps = {
            "xp": nc.dram_tensor("xp", [NP * 128, NH * 512], BF16,
                                 kind="ExternalInput").ap(),
            "wp": nc.dram_tensor("wp", [4 * 128, NH * 256], BF16,
                                 kind="ExternalInput").ap(),
            "gw": nc.dram_tensor("gw", [128, NH * E], BF16,
                                 kind="ExternalInput").ap(),
            "icsr": nc.dram_tensor("icsr", [128, H], F32R,
                                   kind="ExternalInput").ap(),
            "ocs": nc.dram_tensor("ocs", [E, Oc], F32R,
                                  kind="ExternalInput").ap(),
            "bias": nc.dram_tensor("bias", [128, NO], F32,
                                   kind="ExternalInput").ap(),
            "sel": nc.dram_tensor("sel", [128, E], F32R,
                                  kind="ExternalInput").ap(),
            "ones41": nc.dram_tensor("ones41", [E, 1], F32R,
                                     kind="ExternalInput").ap(),
            "ones14": nc.dram_tensor("ones14", [1, E], F32R,
                                     kind="ExternalInput").ap(),
        }
        outs_aps = {"y": nc.dram_tensor("y", [Oc, Nc], F32,
                                        kind="ExternalOutput").ap()}
        with tile.TileContext(nc) as tc:
            with ExitStack() as ctx:
                build_kernel(ctx, tc, outs_aps, ins_aps, CFG)
        nc.compile()
        _NC_CACHE = nc
    return _NC_CACHE


def kernel(x, weight, bias, gate_w, in_channel_scale, out_channel_scale):
    """Full inputs in, full output out; distributes over 8 NeuronCores."""
    global LAST_EXEC_NS, LAST_TRACE_PATH
    from concourse.bass_utils import run_bass_kernel_spmd

    x = np.asarray(x, dtype=np.float32)
    weight = np.asarray(weight, dtype=np.float32)
    bias = np.asarray(bias, dtype=np.float32)
    gate_w = np.asarray(gate_w, dtype=np.float32)
    ics = np.asarray(in_channel_scale, dtype=np.float32)
    ocs = np.asarray(out_channel_scale, dtype=np.float32)

    nc = _get_nc()
    xf = x.reshape(N, H)
    wsign = np.sign(weight).astype(BF)          # [O, H], exactly +-1

    # x per token-half, swizzled: xp[pr*128+p, j*512+t] = x[half, pr*512+t, j*128+p]
    xps = []
    for ti in range(A):
        xh = xf[ti * Nc:(ti + 1) * Nc, :].astype(BF)         # [Nc, H]
        xp = xh.reshape(NP, 512, NH, 128).transpose(0, 3, 2, 1)
        xps.append(np.ascontiguousarray(xp.reshape(NP * 128, NH * 512)))
    # w per o-quarter-slab, swizzled: wp[q*128+p, j*256+c] = sign(w)[oi*1024+q*256+c, j*128+p]
    wps = []
    for oi in range(B):
        ws = wsign[oi * Oc:(oi + 1) * Oc, :]                 # [1024, H]
        wp = ws.reshape(4, 256, NH, 128).transpose(0, 3, 2, 1)
        wps.append(np.ascontiguousarray(wp.reshape(4 * 128, NH * 256)))

    gw_pack = np.ascontiguousarray(
        gate_w.T.reshape(NH, 128, E).transpose(1, 0, 2).reshape(128, NH * E)
    ).astype(BF)
    icsr = np.zeros((128, H), dtype=np.float32)
    selm = np.zeros((128, E), dtype=np.float32)
    for c in range(4):
        icsr[32 * c:32 * c + E, :] = ics
        selm[32 * c + np.arange(E), np.arange(E)] = 1.0
    bias_cols = np.ascontiguousarray(
        bias.reshape(B, NO, 128).transpose(0, 2, 1))         # [B][128, NO]

    in_maps = []
    for c in range(NCORES):
        ti, oi = c // B, c % B
        in_maps.append({
            "xp": xps[ti], "wp": wps[oi], "gw": gw_pack, "icsr": icsr,
            "ocs": np.ascontiguousarray(ocs[:, oi * Oc:(oi + 1) * Oc]),
            "bias": bias_cols[oi], "sel": selm,
            "ones41": np.ones((E, 1), dtype=np.float32),
            "ones14": np.ones((1, E), dtype=np.float32),
        })
    res = run_bass_kernel_spmd(nc, in_maps, core_ids=list(range(NCORES)),
                               trace=TRACE)
    if TRACE:
        LAST_EXEC_NS = res.exec_time_ns
        if res.instructions_and_trace:
            LAST_TRACE_PATH = res.instructions_and_trace[1]
    yfull = np.empty((N, O), dtype=np.float32)
    for c in range(NCORES):
        ti, oi = c // B, c % B
        yfull[ti * Nc:(ti + 1) * Nc, oi * Oc:(oi + 1) * Oc] = \
            res.results[c]["y"].T
    return yfull.reshape(Bsz, S, O)


# revision 19
# speedup vs baseline: 1.2035x; 1.2035x over previous
"""nn_BinaryMoSLinear Trainium2 kernel: 8 NeuronCores, 2 token-halves x 4
out-feature quarters.

kernel(**inputs) takes the FULL reference.setup_inputs() tensors and returns
the FULL [4, 2048, 4096] f32 output. Core c handles token half ti=c//4 and
out-feature quarter oi=c%4 (Nc=4096 tokens, Oc=1024 features per core).

The host pre-binarizes the weight (sign -> bf16, exactly representable) and
casts/transposes x to bf16, so the main matmul runs with a stationary bf16
weight tile [128h x 128o] and a moving activation tile [128h x 512tok] at
1 row/cycle. The token stream is chunked into 8 pairs of 512: while pair k's
256 main matmuls run, pair k+1's router/softmax/in-scale/xs stages are
interleaved into the 8 o-blocks (in-scale matmuls woven into the main
h-loop) so the PE never starves. All K=4 scale matmuls run in bf16 so they
row/col-tile-pack; softmax normalization stays off the PE critical path
(PE broadcasts the exp-sum, DVE does reciprocal+multiply). The xs scaling
multiplies alternate between DVE and the Pool engine. Output is produced
transposed [Oc, Nc] and re-assembled on the host.
"""
from contextlib import ExitStack

import concourse.bass as bass
import concourse.mybir as mybir

F32 = mybir.dt.float32
F32R = mybir.dt.float32r
BF16 = mybir.dt.bfloat16
AF = mybir.ActivationFunctionType
OP = mybir.AluOpType


def f32(ap):
    return ap.bitcast(F32)


def build_kernel(ctx, tc, outs, ins, cfg):
    nc = tc.nc
    H, Oc, Nc, E = cfg["H"], cfg["Oc"], cfg["Nc"], cfg["E"]
    NH = H // 128            # 32 h-chunks
    NO = Oc // 128           # 8 o-tiles
    NP = Nc // 512           # 8 token pairs (tc chunks)
    RING = 3
    y = outs["y"]

    pool = ctx.enter_context(tc.tile_pool(name="sb", bufs=1))

    # ---- persistent SBUF tensors ----
    ring = [pool.tile([128, NH * 512], BF16, name=f"ring{r}", tag=f"ring{r}",
                      bufs=1) for r in range(RING)]
    wq = [pool.tile([128, NH * 256], BF16, name=f"wq{q}", tag=f"wq{q}",
                    bufs=1) for q in range(4)]
    rT_rep = pool.tile([128, Nc], BF16, name="rT_rep", tag="rT", bufs=1)
    ics_rep = pool.tile([128, H], BF16, name="ics_rep", tag="ics", bufs=1)
    gw_pack = pool.tile([128, NH * E], BF16, name="gw_pack", tag="gw", bufs=1)
    ocs_sb = pool.tile([E, Oc], BF16, name="ocs_sb", tag="ocs", bufs=1)
    bias_sb = pool.tile([128, NO], F32, name="bias_sb", tag="bias", bufs=1)
    sel_sb = pool.tile([128, 36], BF16, name="sel_sb", tag="sel", bufs=1)
    ones_e1 = pool.tile([E, 1], BF16, name="ones_e1", tag="o41", bufs=1)
    ones_1e = pool.tile([1, 36], BF16, name="ones_1e", tag="o14", bufs=1)

    # ---- input DMAs: x pairs on the SP hwdge ring, everything else on the
    # ACT ring so the first x pair starts streaming immediately ----
    def x_dma(pr):
        nc.sync.dma_start(ring[pr % RING][:],
                          ins["xp"][pr * 128:(pr + 1) * 128, :])

    for pr in range(min(RING, NP)):
        x_dma(pr)
    nc.scalar.dma_start(gw_pack[:], ins["gw"][:, :])
    nc.scalar.dma_start(sel_sb[:], ins["sel"][:, :])
    nc.scalar.dma_start(ones_e1[:], ins["ones41"][:, :])
    nc.scalar.dma_start(ones_1e[:], ins["ones14"][:, :])
    nc.scalar.dma_start(ocs_sb[:], ins["ocs"][:, :])
    nc.scalar.dma_start(bias_sb[:], ins["bias"][:, :])
    nc.scalar.dma_start(ics_rep[:], ins["icsr"][:, :])
    for q in range(4):
        nc.scalar.dma_start(wq[q][:], ins["wp"][q * 128:(q + 1) * 128, :])

    # ---- PSUM pools (2 + 1 + 2 + 3 = 8 banks) ----
    ps_main = ctx.enter_context(
        tc.tile_pool(name="ps_main", bufs=2, space="PSUM"))
    ps_os = ctx.enter_context(tc.tile_pool(name="ps_os", bufs=1, space="PSUM"))
    ps_sm = ctx.enter_context(tc.tile_pool(name="ps_sm", bufs=2, space="PSUM"))
    ps_is = ctx.enter_context(tc.tile_pool(name="ps_is", bufs=3, space="PSUM"))

    class Prep:
        """Stages preparing pair pr: router -> softmax -> in-scale/xs.

        r (softmax weights) is produced in bf16 on partition strips {0-3}
        and {32-35} so the K=4 in-scale matmuls run 2-row-group packed.
        """

        def __init__(self, pr):
            self.pr = pr
            self.rng = ring[pr % RING]
            self.tsl = slice(pr * 512, (pr + 1) * 512)

        def router(self):
            pr = self.pr
            self.strip = ps_sm.tile([128, 512], F32, name=f"strip{pr}",
                                    tag="psm")
            for r in range(NH // 4):
                for c in range(4):
                    h = 4 * r + c
                    nc.tensor.matmul(
                        self.strip[32 * c:32 * c + E, :],
                        gw_pack[:, h * E:(h + 1) * E],
                        self.rng[:, h * 512:(h + 1) * 512],
                        start=(r == 0), stop=(r == NH // 4 - 1),
                        tile_position=(0, 32 * c))
            sc = pool.tile([128, 512], BF16, name=f"sc{pr}", tag="sc", bufs=2)
            nc.vector.tensor_copy(sc[:], self.strip[:])
            self.sc = sc

        def sm_a(self):
            pr = self.pr
            lg = ps_sm.tile([128, 512], F32, name=f"lg{pr}", tag="psm")
            self.ex = pool.tile([128, 512], BF16, name=f"ex{pr}", tag="ex",
                                bufs=2)
            nc.tensor.matmul(lg[0:36, :], sel_sb[:], self.sc[:],
                             start=True, stop=True)
            for c in range(2):
                nc.scalar.activation(self.ex[32 * c:32 * c + E, :],
                                     lg[32 * c:32 * c + E, :], AF.Exp)

        def sm_b(self):
            pr = self.pr
            ssum = ps_sm.tile([1, 512], F32, name=f"ss{pr}", tag="psm")
            nc.tensor.matmul(ssum[:], ones_e1[:], self.ex[0:E, :],
                             start=True, stop=True)
            self.ss = pool.tile([1, 512], BF16, name=f"ssb{pr}", tag="ssb",
                                bufs=2)
            nc.vector.tensor_copy(self.ss[:], ssum[:])

        def sm_c(self):
            pr = self.pr
            bc = ps_sm.tile([128, 512], F32, name=f"bc{pr}", tag="psm")
            nc.tensor.matmul(bc[0:36, :], ones_1e[:], self.ss[:],
                             start=True, stop=True)
            rcp = pool.tile([128, 512], F32R, name=f"rcp{pr}", tag="rcp",
                            bufs=2)
            with nc.allow_low_precision(reason="softmax reciprocal"):
                nc.vector.reciprocal(rcp[0:36, :], bc[0:36, :])
            for c in range(2):
                s = slice(32 * c, 32 * c + E)
                nc.vector.tensor_tensor(rT_rep[s, self.tsl], self.ex[s, :],
                                        f32(rcp[s, :]), OP.mult)

        def is_round(self, r):
            pr = self.pr
            for c in range(2):
                h = 2 * r + c
                isp = ps_is.tile([128, 512], F32, name=f"is{pr}_{h}",
                                 tag="psi")
                nc.tensor.matmul(
                    isp[:],
                    ics_rep[32 * c:32 * c + E, h * 128:(h + 1) * 128],
                    rT_rep[32 * c:32 * c + E, self.tsl],
                    start=True, stop=True)
                xsl = self.rng[:, h * 512:(h + 1) * 512]
                nc.vector.tensor_tensor(xsl, xsl, isp[:], OP.mult)

        # stage placement inside the consuming TC's o-blocks
        def post(self, o):
            if o == 2:
                self.router()
            elif o == 3:
                self.sm_a()
            elif o == 4:
                self.sm_b()

        def insert(self, o, j):
            if o == 5 and j == 3:
                self.sm_c()
            elif o in (6, 7) and j % 4 == 3:
                self.is_round(8 * (o - 6) + (j - 3) // 4)  # rounds 0..15

    # ---- prologue: router + softmax for pair 0 (its in-scale/xs rounds
    # are woven just-in-time into TC0's first o-block) ----
    prep0 = Prep(0)
    prep0.router()
    prep0.sm_a()
    prep0.sm_b()
    prep0.sm_c()

    # ---- main loop over token pairs ----
    for k in range(NP):
        prep = Prep(k + 1) if k + 1 < NP else None
        rng = ring[k % RING]
        tsl = slice(k * 512, (k + 1) * 512)
        for o in range(NO):
            osp = ps_os.tile([128, 512], F32, name=f"os{k}_{o}", tag="pso")
            nc.tensor.matmul(osp[:], ocs_sb[:, o * 128:(o + 1) * 128],
                             rT_rep[0:E, tsl], start=True, stop=True)
            os_sb = pool.tile([128, 512], F32, name=f"osb{k}_{o}",
                              tag="osb", bufs=2)
            nc.scalar.activation(os_sb[:], osp[:], AF.Copy)
            mp = ps_main.tile([128, 512], F32, name=f"mp{k}_{o}", tag="psm")
            q, half = o // 2, (o % 2) * 128
            for j in range(NH):
                if k == 0 and o == 0 and j % 2 == 0:
                    prep0.is_round(j // 2)          # just-in-time xs, pair 0
                nc.tensor.matmul(
                    mp[:],
                    wq[q][:, j * 256 + half:j * 256 + half + 128],
                    rng[:, j * 512:(j + 1) * 512],
                    start=(j == 0), stop=(j == NH - 1))
                if prep is not None and not (k == 0 and o == 0):
                    prep.insert(o, j)
            if prep is not None:
                prep.post(o)
            y1 = pool.tile([128, 512], F32, name=f"y1_{k}_{o}",
                           tag="y1", bufs=2)
            nc.vector.tensor_tensor(y1[:], mp[:], os_sb[:], OP.mult)
            y2 = pool.tile([128, 512], F32, name=f"y2_{k}_{o}",
                           tag="y2", bufs=3)
            nc.scalar.activation(y2[:], y1[:], AF.Identity,
                                 bias=bias_sb[:, o:o + 1])
            nc.scalar.dma_start(y[o * 128:(o + 1) * 128, tsl], y2[:])
        # refill this ring slot for pair k+RING; emitted after the last
        # reader of the old contents so the overwrite orders correctly
        if k + RING < NP:
            x_dma(k + RING)


import numpy as np
import ml_dtypes

BF = ml_dtypes.bfloat16
NCORES = 8
A, B = 2, 4
Bsz, S, H, O, E = 4, 2048, 4096, 4096, 4
N = Bsz * S
Nc = N // A
Oc = O // B
NH = H // 128
NO = Oc // 128
NP = Nc // 512
CFG = dict(H=H, Oc=Oc, Nc=Nc, E=E)

TRACE = False
LAST_EXEC_NS = None
LAST_TRACE_PATH = None
_NC_CACHE = None


def _get_nc():
    global _NC_CACHE
    if _NC_CACHE is None:
        import concourse.bacc as bacc
        import concourse.tile as tile
        nc = bacc.Bacc("TRN2", target_bir_lowering=False, debug=False,
                       num_devices=NCORES)
        ins_aps = {
            "xp": nc.dram_tensor("xp", [NP * 128, NH * 512], BF16,
                                 kind="ExternalInput").ap(),
            "wp": nc.dram_tensor("wp", [4 * 128, NH * 256], BF16,
                                 kind="ExternalInput").ap(),
            "gw": nc.dram_tensor("gw", [128, NH * E], BF16,
                                 kind="ExternalInput").ap(),
            "icsr": nc.dram_tensor("icsr", [128, H], BF16,
                                   kind="ExternalInput").ap(),
            "ocs": nc.dram_tensor("ocs", [E, Oc], BF16,
                                  kind="ExternalInput").ap(),
            "bias": nc.dram_tensor("bias", [128, NO], F32,
                                   kind="ExternalInput").ap(),
            "sel": nc.dram_tensor("sel", [128, 36], BF16,
                                  kind="ExternalInput").ap(),
            "ones41": nc.dram_tensor("ones41", [E, 1], BF16,
                                     kind="ExternalInput").ap(),
            "ones14": nc.dram_tensor("ones14", [1, 36], BF16,
                                     kind="ExternalInput").ap(),
        }
        outs_aps = {"y": nc.dram_tensor("y", [Oc, Nc], F32,
                                        kind="ExternalOutput").ap()}
        with tile.TileContext(nc) as tc:
            with ExitStack() as ctx:
                build_kernel(ctx, tc, outs_aps, ins_aps, CFG)
        nc.compile()
        _NC_CACHE = nc
    return _NC_CACHE


def kernel(x, weight, bias, gate_w, in_channel_scale, out_channel_scale):
    """Full inputs in, full output out; distributes over 8 NeuronCores."""
    global LAST_EXEC_NS, LAST_TRACE_PATH
    from concourse.bass_utils import run_bass_kernel_spmd

    x = np.asarray(x, dtype=np.float32)
    weight = np.asarray(weight, dtype=np.float32)
    bias = np.asarray(bias, dtype=np.float32)
    gate_w = np.asarray(gate_w, dtype=np.float32)
    ics = np.asarray(in_channel_scale, dtype=np.float32)
    ocs = np.asarray(out_channel_scale, dtype=np.float32)

    nc = _get_nc()
    xf = x.reshape(N, H)
    wsign = np.sign(weight).astype(BF)          # [O, H], exactly +-1

    # x per token-half, swizzled: xp[pr*128+p, j*512+t] = x[half, pr*512+t, j*128+p]
    xps = []
    for ti in range(A):
        xh = xf[ti * Nc:(ti + 1) * Nc, :].astype(BF)         # [Nc, H]
        xp = xh.reshape(NP, 512, NH, 128).transpose(0, 3, 2, 1)
        xps.append(np.ascontiguousarray(xp.reshape(NP * 128, NH * 512)))
    # w per o-quarter-slab, swizzled: wp[q*128+p, j*256+c] = sign(w)[oi*1024+q*256+c, j*128+p]
    wps = []
    for oi in range(B):
        ws = wsign[oi * Oc:(oi + 1) * Oc, :]                 # [1024, H]
        wp = ws.reshape(4, 256, NH, 128).transpose(0, 3, 2, 1)
        wps.append(np.ascontiguousarray(wp.reshape(4 * 128, NH * 256)))

    gw_pack = np.ascontiguousarray(
        gate_w.T.reshape(NH, 128, E).transpose(1, 0, 2).reshape(128, NH * E)
    ).astype(BF)
    icsr = np.zeros((128, H), dtype=np.float32)
    selm = np.zeros((128, 36), dtype=np.float32)
    for c in range(4):
        icsr[32 * c:32 * c + E, :] = ics
        selm[32 * c + np.arange(E), np.arange(E)] = 1.0       # logits strip 0
        selm[32 * c + np.arange(E), 32 + np.arange(E)] = 1.0  # logits strip 1
    bias_cols = np.ascontiguousarray(
        bias.reshape(B, NO, 128).transpose(0, 2, 1))         # [B][128, NO]

    in_maps = []
    for c in range(NCORES):
        ti, oi = c // B, c % B
        in_maps.append({
            "xp": xps[ti], "wp": wps[oi], "gw": gw_pack,
            "icsr": icsr.astype(BF),
            "ocs": np.ascontiguousarray(ocs[:, oi * Oc:(oi + 1) * Oc]).astype(BF),
            "bias": bias_cols[oi], "sel": selm.astype(BF),
            "ones41": np.ones((E, 1), dtype=np.float32).astype(BF),
            "ones14": np.ones((1, 36), dtype=np.float32).astype(BF),
        })
    res = run_bass_kernel_spmd(nc, in_maps, core_ids=list(range(NCORES)),
                               trace=TRACE)
    if TRACE:
        LAST_EXEC_NS = res.exec_time_ns
        if res.instructions_and_trace:
            LAST_TRACE_PATH = res.instructions_and_trace[1]
    yfull = np.empty((N, O), dtype=np.float32)
    for c in range(NCORES):
        ti, oi = c // B, c % B
        yfull[ti * Nc:(ti + 1) * Nc, oi * Oc:(oi + 1) * Oc] = \
            res.results[c]["y"].T
    return yfull.reshape(Bsz, S, O)


# revision 30
# speedup vs baseline: 1.2147x; 1.0093x over previous
"""nn_BinaryMoSLinear Trainium2 kernel: 8 NeuronCores, 2 token-halves x 4
out-feature quarters.

kernel(**inputs) takes the FULL reference.setup_inputs() tensors and returns
the FULL [4, 2048, 4096] f32 output. Core c handles token half ti=c//4 and
out-feature quarter oi=c%4 (Nc=4096 tokens, Oc=1024 features per core).

The host pre-binarizes the weight (sign -> bf16, exactly representable) and
casts/transposes x to bf16, so the main matmul runs with a stationary bf16
weight tile [128h x 128o] and a moving activation tile [128h x 512tok] at
1 row/cycle. The token stream is chunked into 8 pairs of 512: while pair k's
256 main matmuls run, pair k+1's router/softmax/in-scale/xs stages are
interleaved into the 8 o-blocks (in-scale matmuls woven into the main
h-loop) so the PE never starves. All K=4 scale matmuls run in bf16 so they
row/col-tile-pack; softmax normalization stays off the PE critical path
(PE broadcasts the exp-sum, DVE does reciprocal+multiply). The xs scaling
multiplies alternate between DVE and the Pool engine. Output is produced
transposed [Oc, Nc] and re-assembled on the host.
"""
from contextlib import ExitStack

import concourse.bass as bass
import concourse.mybir as mybir

F32 = mybir.dt.float32
F32R = mybir.dt.float32r
BF16 = mybir.dt.bfloat16
AF = mybir.ActivationFunctionType
OP = mybir.AluOpType


def f32(ap):
    return ap.bitcast(F32)


def build_kernel(ctx, tc, outs, ins, cfg):
    nc = tc.nc
    H, Oc, Nc, E = cfg["H"], cfg["Oc"], cfg["Nc"], cfg["E"]
    NH = H // 128            # 32 h-chunks
    NO = Oc // 128           # 8 o-tiles
    NP = Nc // 512           # 8 token pairs (tc chunks)
    RING = 3
    y = outs["y"]

    pool = ctx.enter_context(tc.tile_pool(name="sb", bufs=1))

    # ---- persistent SBUF tensors ----
    ring = [pool.tile([128, NH * 512], BF16, name=f"ring{r}", tag=f"ring{r}",
                      bufs=1) for r in range(RING)]
    wq = [pool.tile([128, NH * 256], BF16, name=f"wq{q}", tag=f"wq{q}",
                    bufs=1) for q in range(4)]
    rT_rep = pool.tile([128, Nc], BF16, name="rT_rep", tag="rT", bufs=1)
    ics_rep = pool.tile([128, H], BF16, name="ics_rep", tag="ics", bufs=1)
    gw_pack = pool.tile([128, NH * E], BF16, name="gw_pack", tag="gw", bufs=1)
    ocs_sb = pool.tile([E, Oc], BF16, name="ocs_sb", tag="ocs", bufs=1)
    bias_sb = pool.tile([128, NO], F32, name="bias_sb", tag="bias", bufs=1)
    sel_sb = pool.tile([128, 100], BF16, name="sel_sb", tag="sel", bufs=1)
    ones_e1 = pool.tile([E, 1], BF16, name="ones_e1", tag="o41", bufs=1)
    ones_1e = pool.tile([1, 100], BF16, name="ones_1e", tag="o14", bufs=1)

    # ---- input DMAs: x pairs on the SP hwdge ring, everything else on the
    # ACT ring so the first x pair starts streaming immediately ----
    def x_dma(pr):
        nc.sync.dma_start(ring[pr % RING][:],
                          ins["xp"][pr * 128:(pr + 1) * 128, :])

    # pair 0 in two halves so the router can start on the first half
    hw = NH // 2 * 512
    nc.sync.dma_start(ring[0][:, 0:hw], ins["xp"][0:128, 0:hw])
    nc.sync.dma_start(ring[0][:, hw:], ins["xp"][0:128, hw:])
    for pr in range(1, min(RING, NP)):
        x_dma(pr)
    nc.scalar.dma_start(gw_pack[:], ins["gw"][:, :])
    nc.scalar.dma_start(sel_sb[:], ins["sel"][:, :])
    nc.scalar.dma_start(ones_e1[:], ins["ones41"][:, :])
    nc.scalar.dma_start(ones_1e[:], ins["ones14"][:, :])
    nc.scalar.dma_start(ocs_sb[:], ins["ocs"][:, :])
    nc.scalar.dma_start(bias_sb[:], ins["bias"][:, :])
    nc.scalar.dma_start(ics_rep[:], ins["icsr"][:, :])
    for q in range(4):
        nc.scalar.dma_start(wq[q][:], ins["wp"][q * 128:(q + 1) * 128, :])

    # ---- PSUM pools (2 + 1 + 1 + 4 = 8 banks) ----
    ps_main = ctx.enter_context(
        tc.tile_pool(name="ps_main", bufs=2, space="PSUM"))
    ps_os = ctx.enter_context(tc.tile_pool(name="ps_os", bufs=1, space="PSUM"))
    ps_sm = ctx.enter_context(tc.tile_pool(name="ps_sm", bufs=1, space="PSUM"))
    ps_is = ctx.enter_context(tc.tile_pool(name="ps_is", bufs=4, space="PSUM"))

    class Prep:
        """Stages preparing pair pr: router -> softmax -> in-scale/xs.

        r (softmax weights) is produced in bf16 on partition strips {0-3}
        and {32-35} so the K=4 in-scale matmuls run 2-row-group packed.
        """

        def __init__(self, pr):
            self.pr = pr
            self.rng = ring[pr % RING]
            self.tsl = slice(pr * 512, (pr + 1) * 512)

        def router(self):
            pr = self.pr
            self.strip = ps_sm.tile([128, 512], F32, name=f"strip{pr}",
                                    tag="psm")
            for r in range(NH // 4):
                for c in range(4):
                    h = 4 * r + c
                    nc.tensor.matmul(
                        self.strip[32 * c:32 * c + E, :],
                        gw_pack[:, h * E:(h + 1) * E],
                        self.rng[:, h * 512:(h + 1) * 512],
                        start=(r == 0), stop=(r == NH // 4 - 1),
                        tile_position=(0, 32 * c))
            sc = pool.tile([128, 512], BF16, name=f"sc{pr}", tag="sc", bufs=2)
            nc.vector.tensor_copy(sc[:], self.strip[:])
            self.sc = sc

        def sm_a(self):
            pr = self.pr
            lg = ps_sm.tile([128, 512], F32, name=f"lg{pr}", tag="psm")
            self.ex = pool.tile([128, 512], BF16, name=f"ex{pr}", tag="ex",
                                bufs=2)
            nc.tensor.matmul(lg[0:100, :], sel_sb[:], self.sc[:],
                             start=True, stop=True)
            for c in range(4):
                nc.scalar.activation(self.ex[32 * c:32 * c + E, :],
                                     lg[32 * c:32 * c + E, :], AF.Exp)

        def sm_b(self):
            pr = self.pr
            ssum = ps_sm.tile([1, 512], F32, name=f"ss{pr}", tag="psm")
            nc.tensor.matmul(ssum[:], ones_e1[:], self.ex[0:E, :],
                             start=True, stop=True)
            self.ss = pool.tile([1, 512], BF16, name=f"ssb{pr}", tag="ssb",
                                bufs=2)
            nc.vector.tensor_copy(self.ss[:], ssum[:])

        def sm_c(self):
            pr = self.pr
            bc = ps_sm.tile([128, 512], F32, name=f"bc{pr}", tag="psm")
            nc.tensor.matmul(bc[0:100, :], ones_1e[:], self.ss[:],
                             start=True, stop=True)
            rcp = pool.tile([128, 512], F32R, name=f"rcp{pr}", tag="rcp",
                            bufs=2)
            with nc.allow_low_precision(reason="softmax reciprocal"):
                nc.vector.reciprocal(rcp[0:100, :], bc[0:100, :])
            for c in range(4):
                s = slice(32 * c, 32 * c + E)
                nc.vector.tensor_tensor(rT_rep[s, self.tsl], self.ex[s, :],
                                        f32(rcp[s, :]), OP.mult)

        def is_quad(self, r):
            """4-row-group packed in-scale matmuls for h = 4r..4r+3."""
            pr = self.pr
            isps = []
            for c in range(4):
                h = 4 * r + c
                isp = ps_is.tile([128, 512], F32, name=f"is{pr}_{h}",
                                 tag="psi")
                nc.tensor.matmul(
                    isp[:],
                    ics_rep[32 * c:32 * c + E, h * 128:(h + 1) * 128],
                    rT_rep[32 * c:32 * c + E, self.tsl],
                    start=True, stop=True, tile_position=(32 * c, 0))
                isps.append(isp)
            for c in range(4):
                h = 4 * r + c
                xsl = self.rng[:, h * 512:(h + 1) * 512]
                nc.vector.tensor_tensor(xsl, xsl, isps[c][:], OP.mult)

        # stage placement inside the consuming TC's o-blocks
        def post(self, o):
            if o == 2:
                self.router()
            elif o == 3:
                self.sm_a()
            elif o == 4:
                self.sm_b()

        def insert(self, o, j):
            if o == 5 and j == 3:
                self.sm_c()
            elif o in (6, 7) and j % 8 == 3:
                self.is_quad(4 * (o - 6) + (j - 3) // 8)  # quads 0..7

    # ---- prologue: router + softmax for pair 0 (its in-scale/xs quads
    # are woven just-in-time into TC0's first o-block) ----
    prep0 = Prep(0)
    prep0.router()
    prep0.sm_a()
    # ~6us of dense dummy matmuls: forces the PE HAM clock-gate to 8/8
    # before TC0 so the DVE-paced warmup region doesn't run at half clock.
    dummy = ps_is.tile([E, 128], F32, name="dummy", tag="psi")
    for i in range(96):
        nc.tensor.matmul(dummy[:], gw_pack[:, 0:E], gw_pack[:, 0:128],
                         start=True, stop=True)
    prep0.sm_b()
    prep0.sm_c()

    # ---- main loop over token pairs ----
    for k in range(NP):
        prep = Prep(k + 1) if k + 1 < NP else None
        rng = ring[k % RING]
        tsl = slice(k * 512, (k + 1) * 512)
        for o in range(NO):
            osp = ps_os.tile([128, 512], F32, name=f"os{k}_{o}", tag="pso")
            nc.tensor.matmul(osp[:], ocs_sb[:, o * 128:(o + 1) * 128],
                             rT_rep[0:E, tsl], start=True, stop=True)
            os_sb = pool.tile([128, 512], F32, name=f"osb{k}_{o}",
                              tag="osb", bufs=2)
            nc.scalar.activation(os_sb[:], osp[:], AF.Copy)
            mp = ps_main.tile([128, 512], F32, name=f"mp{k}_{o}", tag="psm")
            q, half = o // 2, (o % 2) * 128
            for j in range(NH):
                if k == 0 and o == 0 and j % 4 == 0:
                    prep0.is_quad(j // 4)           # just-in-time xs, pair 0
                nc.tensor.matmul(
                    mp[:],
                    wq[q][:, j * 256 + half:j * 256 + half + 128],
                    rng[:, j * 512:(j + 1) * 512],
                    start=(j == 0), stop=(j == NH - 1))
                if prep is not None and not (k == 0 and o == 0):
                    prep.insert(o, j)
            if prep is not None:
                prep.post(o)
            y1 = pool.tile([128, 512], F32, name=f"y1_{k}_{o}",
                           tag="y1", bufs=2)
            nc.vector.tensor_tensor(y1[:], mp[:], os_sb[:], OP.mult)
            y2 = pool.tile([128, 512], F32, name=f"y2_{k}_{o}",
                           tag="y2", bufs=3)
            nc.scalar.activation(y2[:], y1[:], AF.Identity,
                                 bias=bias_sb[:, o:o + 1])
            nc.scalar.dma_start(y[o * 128:(o + 1) * 128, tsl], y2[:])
        # refill this ring slot for pair k+RING; emitted after the last
        # reader of the old contents so the overwrite orders correctly
        if k + RING < NP:
            x_dma(k + RING)


import numpy as np
import ml_dtypes

BF = ml_dtypes.bfloat16
NCORES = 8
A, B = 2, 4
Bsz, S, H, O, E = 4, 2048, 4096, 4096, 4
N = Bsz * S
Nc = N // A
Oc = O // B
NH = H // 128
NO = Oc // 128
NP = Nc // 512
CFG = dict(H=H, Oc=Oc, Nc=Nc, E=E)

TRACE = False
LAST_EXEC_NS = None
LAST_TRACE_PATH = None
_NC_CACHE = None


def _get_nc():
    global _NC_CACHE
    if _NC_CACHE is None:
        import concourse.bacc as bacc
        import concourse.tile as tile
        nc = bacc.Bacc("TRN2", target_bir_lowering=False, debug=False,
                       num_devices=NCORES)
        ins_aps = {
            "xp": nc.dram_tensor("xp", [NP * 128, NH * 512], BF16,
                                 kind="ExternalInput").ap(),
            "wp": nc.dram_tensor("wp", [4 * 128, NH * 256], BF16,
                                 kind="ExternalInput").ap(),
            "gw": nc.dram_tensor("gw", [128, NH * E], BF16,
                                 kind="ExternalInput").ap(),
            "icsr": nc.dram_tensor("icsr", [128, H], BF16,
                                   kind="ExternalInput").ap(),
            "ocs": nc.dram_tensor("ocs", [E, Oc], BF16,
                                  kind="ExternalInput").ap(),
            "bias": nc.dram_tensor("bias", [128, NO], F32,
                                   kind="ExternalInput").ap(),
            "sel": nc.dram_tensor("sel", [128, 100], BF16,
                                  kind="ExternalInput").ap(),
            "ones41": nc.dram_tensor("ones41", [E, 1], BF16,
                                     kind="ExternalInput").ap(),
            "ones14": nc.dram_tensor("ones14", [1, 100], BF16,
                                     kind="ExternalInput").ap(),
        }
        outs_aps = {"y": nc.dram_tensor("y", [Oc, Nc], F32,
                                        kind="ExternalOutput").ap()}
        with tile.TileContext(nc) as tc:
            with ExitStack() as ctx:
                build_kernel(ctx, tc, outs_aps, ins_aps, CFG)
        nc.compile()
        _NC_CACHE = nc
    return _NC_CACHE


def kernel(x, weight, bias, gate_w, in_channel_scale, out_channel_scale):
    """Full inputs in, full output out; distributes over 8 NeuronCores."""
    global LAST_EXEC_NS, LAST_TRACE_PATH
    from concourse.bass_utils import run_bass_kernel_spmd

    x = np.asarray(x, dtype=np.float32)
    weight = np.asarray(weight, dtype=np.float32)
    bias = np.asarray(bias, dtype=np.float32)
    gate_w = np.asarray(gate_w, dtype=np.float32)
    ics = np.asarray(in_channel_scale, dtype=np.float32)
    ocs = np.asarray(out_channel_scale, dtype=np.float32)

    nc = _get_nc()
    xf = x.reshape(N, H)
    wsign = np.sign(weight).astype(BF)          # [O, H], exactly +-1

    # x per token-half, swizzled: xp[pr*128+p, j*512+t] = x[half, pr*512+t, j*128+p]
    xps = []
    for ti in range(A):
        xh = xf[ti * Nc:(ti + 1) * Nc, :].astype(BF)         # [Nc, H]
        xp = xh.reshape(NP, 512, NH, 128).transpose(0, 3, 2, 1)
        xps.append(np.ascontiguousarray(xp.reshape(NP * 128, NH * 512)))
    # w per o-quarter-slab, swizzled: wp[q*128+p, j*256+c] = sign(w)[oi*1024+q*256+c, j*128+p]
    wps = []
    for oi in range(B):
        ws = wsign[oi * Oc:(oi + 1) * Oc, :]                 # [1024, H]
        wp = ws.reshape(4, 256, NH, 128).transpose(0, 3, 2, 1)
        wps.append(np.ascontiguousarray(wp.reshape(4 * 128, NH * 256)))

    gw_pack = np.ascontiguousarray(
        gate_w.T.reshape(NH, 128, E).transpose(1, 0, 2).reshape(128, NH * E)
    ).astype(BF)
    icsr = np.zeros((128, H), dtype=np.float32)
    selm = np.zeros((128, 100), dtype=np.float32)
    for c in range(4):
        icsr[32 * c:32 * c + E, :] = ics
        for co in range(4):   # replicate logits onto 4 partition strips
            selm[32 * c + np.arange(E), 32 * co + np.arange(E)] = 1.0
    bias_cols = np.ascontiguousarray(
        bias.reshape(B, NO, 128).transpose(0, 2, 1))         # [B][128, NO]

    in_maps = []
    for c in range(NCORES):
        ti, oi = c // B, c % B
        in_maps.append({
            "xp": xps[ti], "wp": wps[oi], "gw": gw_pack,
            "icsr": icsr.astype(BF),
            "ocs": np.ascontiguousarray(ocs[:, oi * Oc:(oi + 1) * Oc]).astype(BF),
            "bias": bias_cols[oi], "sel": selm.astype(BF),
            "ones41": np.ones((E, 1), dtype=np.float32).astype(BF),
            "ones14": np.ones((1, 100), dtype=np.float32).astype(BF),
        })
    res = run_bass_kernel_spmd(nc, in_maps, core_ids=list(range(NCORES)),
                               trace=TRACE)
    if TRACE:
        LAST_EXEC_NS = res.exec_time_ns
        if res.instructions_and_trace:
            LAST_TRACE_PATH = res.instructions_and_trace[1]
    yfull = np.empty((N, O), dtype=np.float32)
    for c in range(NCORES):
        ti, oi = c // B, c % B
        yfull[ti * Nc:(ti + 1) * Nc, oi * Oc:(oi + 1) * Oc] = \
            res.results[c]["y"].T
    return yfull.reshape(Bsz, S, O)


# revision 31
# speedup vs baseline: 1.2204x; 1.0047x over previous
"""nn_BinaryMoSLinear Trainium2 kernel: 8 NeuronCores, 2 token-halves x 4
out-feature quarters.

kernel(**inputs) takes the FULL reference.setup_inputs() tensors and returns
the FULL [4, 2048, 4096] f32 output. Core c handles token half ti=c//4 and
out-feature quarter oi=c%4 (Nc=4096 tokens, Oc=1024 features per core).

The host pre-binarizes the weight (sign -> bf16, exactly representable) and
casts/transposes x to bf16, so the main matmul runs with a stationary bf16
weight tile [128h x 128o] and a moving activation tile [128h x 512tok] at
1 row/cycle. The token stream is chunked into 8 pairs of 512: while pair k's
256 main matmuls run, pair k+1's router/softmax/in-scale/xs stages are
interleaved into the 8 o-blocks (in-scale matmuls woven into the main
h-loop) so the PE never starves. All K=4 scale matmuls run in bf16 so they
row/col-tile-pack; softmax normalization stays off the PE critical path
(PE broadcasts the exp-sum, DVE does reciprocal+multiply). The xs scaling
multiplies alternate between DVE and the Pool engine. Output is produced
transposed [Oc, Nc] and re-assembled on the host.
"""
from contextlib import ExitStack

import concourse.bass as bass
import concourse.mybir as mybir

F32 = mybir.dt.float32
F32R = mybir.dt.float32r
BF16 = mybir.dt.bfloat16
AF = mybir.ActivationFunctionType
OP = mybir.AluOpType


def f32(ap):
    return ap.bitcast(F32)


def build_kernel(ctx, tc, outs, ins, cfg):
    nc = tc.nc
    H, Oc, Nc, E = cfg["H"], cfg["Oc"], cfg["Nc"], cfg["E"]
    NH = H // 128            # 32 h-chunks
    NO = Oc // 128           # 8 o-tiles
    NP = Nc // 512           # 8 token pairs (tc chunks)
    RING = 3
    y = outs["y"]

    pool = ctx.enter_context(tc.tile_pool(name="sb", bufs=1))

    # ---- persistent SBUF tensors ----
    ring = [pool.tile([128, NH * 512], BF16, name=f"ring{r}", tag=f"ring{r}",
                      bufs=1) for r in range(RING)]
    wq = [pool.tile([128, NH * 256], BF16, name=f"wq{q}", tag=f"wq{q}",
                    bufs=1) for q in range(4)]
    rT_rep = pool.tile([128, Nc], BF16, name="rT_rep", tag="rT", bufs=1)
    ics_rep = pool.tile([128, H], BF16, name="ics_rep", tag="ics", bufs=1)
    gw_pack = pool.tile([128, NH * E], BF16, name="gw_pack", tag="gw", bufs=1)
    ocs_sb = pool.tile([E, Oc], BF16, name="ocs_sb", tag="ocs", bufs=1)
    bias_sb = pool.tile([128, NO], F32, name="bias_sb", tag="bias", bufs=1)
    sel_sb = pool.tile([128, 100], BF16, name="sel_sb", tag="sel", bufs=1)
    ones_e1 = pool.tile([E, 1], BF16, name="ones_e1", tag="o41", bufs=1)
    ones_1e = pool.tile([1, 100], BF16, name="ones_1e", tag="o14", bufs=1)

    # ---- input DMAs: x pairs on the SP hwdge ring, everything else on the
    # ACT ring so the first x pair starts streaming immediately ----
    def x_dma(pr):
        nc.sync.dma_start(ring[pr % RING][:],
                          ins["xp"][pr * 128:(pr + 1) * 128, :])

    # pair 0 in two halves so the router can start on the first half
    hw = NH // 2 * 512
    nc.sync.dma_start(ring[0][:, 0:hw], ins["xp"][0:128, 0:hw])
    nc.sync.dma_start(ring[0][:, hw:], ins["xp"][0:128, hw:])
    for pr in range(1, min(RING, NP)):
        x_dma(pr)
    nc.scalar.dma_start(gw_pack[:], ins["gw"][:, :])
    nc.scalar.dma_start(sel_sb[:], ins["sel"][:, :])
    nc.scalar.dma_start(ones_e1[:], ins["ones41"][:, :])
    nc.scalar.dma_start(ones_1e[:], ins["ones14"][:, :])
    nc.scalar.dma_start(ocs_sb[:], ins["ocs"][:, :])
    nc.scalar.dma_start(bias_sb[:], ins["bias"][:, :])
    nc.scalar.dma_start(ics_rep[:], ins["icsr"][:, :])
    for q in range(4):
        nc.scalar.dma_start(wq[q][:], ins["wp"][q * 128:(q + 1) * 128, :])

    # ---- PSUM pools (2 + 1 + 1 + 4 = 8 banks) ----
    ps_main = ctx.enter_context(
        tc.tile_pool(name="ps_main", bufs=2, space="PSUM"))
    ps_os = ctx.enter_context(tc.tile_pool(name="ps_os", bufs=1, space="PSUM"))
    ps_sm = ctx.enter_context(tc.tile_pool(name="ps_sm", bufs=1, space="PSUM"))
    ps_is = ctx.enter_context(tc.tile_pool(name="ps_is", bufs=4, space="PSUM"))

    class Prep:
        """Stages preparing pair pr: router -> softmax -> in-scale/xs.

        r (softmax weights) is produced in bf16 on partition strips {0-3}
        and {32-35} so the K=4 in-scale matmuls run 2-row-group packed.
        """

        def __init__(self, pr):
            self.pr = pr
            self.rng = ring[pr % RING]
            self.tsl = slice(pr * 512, (pr + 1) * 512)

        def router(self):
            pr = self.pr
            self.strip = ps_sm.tile([128, 512], F32, name=f"strip{pr}",
                                    tag="psm")
            for r in range(NH // 4):
                for c in range(4):
                    h = 4 * r + c
                    nc.tensor.matmul(
                        self.strip[32 * c:32 * c + E, :],
                        gw_pack[:, h * E:(h + 1) * E],
                        self.rng[:, h * 512:(h + 1) * 512],
                        start=(r == 0), stop=(r == NH // 4 - 1),
                        tile_position=(0, 32 * c))
            sc = pool.tile([128, 512], BF16, name=f"sc{pr}", tag="sc", bufs=2)
            nc.vector.tensor_copy(sc[:], self.strip[:])
            self.sc = sc

        def sm_a(self):
            pr = self.pr
            lg = ps_sm.tile([128, 512], F32, name=f"lg{pr}", tag="psm")
            self.ex = pool.tile([128, 512], BF16, name=f"ex{pr}", tag="ex",
                                bufs=2)
            nc.tensor.matmul(lg[0:100, :], sel_sb[:], self.sc[:],
                             start=True, stop=True)
            for c in range(4):
                nc.scalar.activation(self.ex[32 * c:32 * c + E, :],
                                     lg[32 * c:32 * c + E, :], AF.Exp)

        def sm_b(self):
            pr = self.pr
            ssum = ps_sm.tile([1, 512], F32, name=f"ss{pr}", tag="psm")
            nc.tensor.matmul(ssum[:], ones_e1[:], self.ex[0:E, :],
                             start=True, stop=True)
            self.ss = pool.tile([1, 512], BF16, name=f"ssb{pr}", tag="ssb",
                                bufs=2)
            nc.vector.tensor_copy(self.ss[:], ssum[:])

        def sm_c(self):
            pr = self.pr
            bc = ps_sm.tile([128, 512], F32, name=f"bc{pr}", tag="psm")
            nc.tensor.matmul(bc[0:100, :], ones_1e[:], self.ss[:],
                             start=True, stop=True)
            rcp = pool.tile([128, 512], F32R, name=f"rcp{pr}", tag="rcp",
                            bufs=2)
            with nc.allow_low_precision(reason="softmax reciprocal"):
                nc.vector.reciprocal(rcp[0:100, :], bc[0:100, :])
            for c in range(4):
                s = slice(32 * c, 32 * c + E)
                nc.vector.tensor_tensor(rT_rep[s, self.tsl], self.ex[s, :],
                                        f32(rcp[s, :]), OP.mult)

        def is_quad(self, r):
            """4-row-group packed in-scale matmuls for h = 4r..4r+3."""
            pr = self.pr
            isps = []
            for c in range(4):
                h = 4 * r + c
                isp = ps_is.tile([128, 512], F32, name=f"is{pr}_{h}",
                                 tag="psi")
                nc.tensor.matmul(
                    isp[:],
                    ics_rep[32 * c:32 * c + E, h * 128:(h + 1) * 128],
                    rT_rep[32 * c:32 * c + E, self.tsl],
                    start=True, stop=True, tile_position=(32 * c, 0))
                isps.append(isp)
            for c in range(4):
                h = 4 * r + c
                xsl = self.rng[:, h * 512:(h + 1) * 512]
                nc.vector.tensor_tensor(xsl, xsl, isps[c][:], OP.mult)

        # stage placement inside the consuming TC's o-blocks
        def post(self, o):
            if o == 2:
                self.router()
            elif o == 3:
                self.sm_a()
            elif o == 4:
                self.sm_b()

        def insert(self, o, j):
            if o == 5 and j == 3:
                self.sm_c()
            elif o in (6, 7) and j % 8 == 3:
                self.is_quad(4 * (o - 6) + (j - 3) // 8)  # quads 0..7

    # ---- prologue: router + softmax for pair 0 (its in-scale/xs quads
    # are woven just-in-time into TC0's first o-block) ----
    prep0 = Prep(0)
    prep0.router()
    prep0.sm_a()
    # ~6us of dense dummy matmuls: forces the PE HAM clock-gate to 8/8
    # before TC0 so the DVE-paced warmup region doesn't run at half clock.
    dummy = ps_is.tile([E, 128], F32, name="dummy", tag="psi")
    for i in range(96):
        nc.tensor.matmul(dummy[:], gw_pack[:, 0:E], gw_pack[:, 0:128],
                         start=True, stop=True)
    prep0.sm_b()
    prep0.sm_c()
    # second burst: keep the PE busy (and the clock-gate open) while the
    # DVE runs pair 0's reciprocal + r multiplies
    dummy2 = ps_is.tile([E, 128], F32, name="dummy2", tag="psi")
    for i in range(96):
        nc.tensor.matmul(dummy2[:], gw_pack[:, 0:E], gw_pack[:, 0:128],
                         start=True, stop=True)

    # ---- main loop over token pairs ----
    for k in range(NP):
        prep = Prep(k + 1) if k + 1 < NP else None
        rng = ring[k % RING]
        tsl = slice(k * 512, (k + 1) * 512)
        for o in range(NO):
            osp = ps_os.tile([128, 512], F32, name=f"os{k}_{o}", tag="pso")
            nc.tensor.matmul(osp[:], ocs_sb[:, o * 128:(o + 1) * 128],
                             rT_rep[0:E, tsl], start=True, stop=True)
            os_sb = pool.tile([128, 512], F32, name=f"osb{k}_{o}",
                              tag="osb", bufs=2)
            nc.scalar.activation(os_sb[:], osp[:], AF.Copy)
            mp = ps_main.tile([128, 512], F32, name=f"mp{k}_{o}", tag="psm")
            q, half = o // 2, (o % 2) * 128
            for j in range(NH):
                if k == 0 and o == 0 and j % 4 == 0:
                    prep0.is_quad(j // 4)           # just-in-time xs, pair 0
                nc.tensor.matmul(
                    mp[:],
                    wq[q][:, j * 256 + half:j * 256 + half + 128],
                    rng[:, j * 512:(j + 1) * 512],
                    start=(j == 0), stop=(j == NH - 1))
                if prep is not None and not (k == 0 and o == 0):
                    prep.insert(o, j)
            if prep is not None:
                prep.post(o)
            y1 = pool.tile([128, 512], F32, name=f"y1_{k}_{o}",
                           tag="y1", bufs=2)
            nc.vector.tensor_tensor(y1[:], mp[:], os_sb[:], OP.mult)
            y2 = pool.tile([128, 512], F32, name=f"y2_{k}_{o}",
                           tag="y2", bufs=3)
            nc.scalar.activation(y2[:], y1[:], AF.Identity,
                                 bias=bias_sb[:, o:o + 1])
            nc.scalar.dma_start(y[o * 128:(o + 1) * 128, tsl], y2[:])
        # refill this ring slot for pair k+RING; emitted after the last
        # reader of the old contents so the overwrite orders correctly
        if k + RING < NP:
            x_dma(k + RING)


import numpy as np
import ml_dtypes

BF = ml_dtypes.bfloat16
NCORES = 8
A, B = 2, 4
Bsz, S, H, O, E = 4, 2048, 4096, 4096, 4
N = Bsz * S
Nc = N // A
Oc = O // B
NH = H // 128
NO = Oc // 128
NP = Nc // 512
CFG = dict(H=H, Oc=Oc, Nc=Nc, E=E)

TRACE = False
LAST_EXEC_NS = None
LAST_TRACE_PATH = None
_NC_CACHE = None


def _get_nc():
    global _NC_CACHE
    if _NC_CACHE is None:
        import concourse.bacc as bacc
        import concourse.tile as tile
        nc = bacc.Bacc("TRN2", target_bir_lowering=False, debug=False,
                       num_devices=NCORES)
        ins_aps = {
            "xp": nc.dram_tensor("xp", [NP * 128, NH * 512], BF16,
                                 kind="ExternalInput").ap(),
            "wp": nc.dram_tensor("wp", [4 * 128, NH * 256], BF16,
                                 kind="ExternalInput").ap(),
            "gw": nc.dram_tensor("gw", [128, NH * E], BF16,
                                 kind="ExternalInput").ap(),
            "icsr": nc.dram_tensor("icsr", [128, H], BF16,
                                   kind="ExternalInput").ap(),
            "ocs": nc.dram_tensor("ocs", [E, Oc], BF16,
                                  kind="ExternalInput").ap(),
            "bias": nc.dram_tensor("bias", [128, NO], F32,
                                   kind="ExternalInput").ap(),
            "sel": nc.dram_tensor("sel", [128, 100], BF16,
                                  kind="ExternalInput").ap(),
            "ones41": nc.dram_tensor("ones41", [E, 1], BF16,
                                     kind="ExternalInput").ap(),
            "ones14": nc.dram_tensor("ones14", [1, 100], BF16,
                                     kind="ExternalInput").ap(),
        }
        outs_aps = {"y": nc.dram_tensor("y", [Oc, Nc], F32,
                                        kind="ExternalOutput").ap()}
        with tile.TileContext(nc) as tc:
            with ExitStack() as ctx:
                build_kernel(ctx, tc, outs_aps, ins_aps, CFG)
        nc.compile()
        _NC_CACHE = nc
    return _NC_CACHE


def kernel(x, weight, bias, gate_w, in_channel_scale, out_channel_scale):
    """Full inputs in, full output out; distributes over 8 NeuronCores."""
    global LAST_EXEC_NS, LAST_TRACE_PATH
    from concourse.bass_utils import run_bass_kernel_spmd

    x = np.asarray(x, dtype=np.float32)
    weight = np.asarray(weight, dtype=np.float32)
    bias = np.asarray(bias, dtype=np.float32)
    gate_w = np.asarray(gate_w, dtype=np.float32)
    ics = np.asarray(in_channel_scale, dtype=np.float32)
    ocs = np.asarray(out_channel_scale, dtype=np.float32)

    nc = _get_nc()
    xf = x.reshape(N, H)
    wsign = np.sign(weight).astype(BF)          # [O, H], exactly +-1

    # x per token-half, swizzled: xp[pr*128+p, j*512+t] = x[half, pr*512+t, j*128+p]
    xps = []
    for ti in range(A):
        xh = xf[ti * Nc:(ti + 1) * Nc, :].astype(BF)         # [Nc, H]
        xp = xh.reshape(NP, 512, NH, 128).transpose(0, 3, 2, 1)
        xps.append(np.ascontiguousarray(xp.reshape(NP * 128, NH * 512)))
    # w per o-quarter-slab, swizzled: wp[q*128+p, j*256+c] = sign(w)[oi*1024+q*256+c, j*128+p]
    wps = []
    for oi in range(B):
        ws = wsign[oi * Oc:(oi + 1) * Oc, :]                 # [1024, H]
        wp = ws.reshape(4, 256, NH, 128).transpose(0, 3, 2, 1)
        wps.append(np.ascontiguousarray(wp.reshape(4 * 128, NH * 256)))

    gw_pack = np.ascontiguousarray(
        gate_w.T.reshape(NH, 128, E).transpose(1, 0, 2).reshape(128, NH * E)
    ).astype(BF)
    icsr = np.zeros((128, H), dtype=np.float32)
    selm = np.zeros((128, 100), dtype=np.float32)
    for c in range(4):
        icsr[32 * c:32 * c + E, :] = ics
        for co in range(4):   # replicate logits onto 4 partition strips
            selm[32 * c + np.arange(E), 32 * co + np.arange(E)] = 1.0
    bias_cols = np.ascontiguousarray(
        bias.reshape(B, NO, 128).transpose(0, 2, 1))         # [B][128, NO]

    in_maps = []
    for c in range(NCORES):
        ti, oi = c // B, c % B
        in_maps.append({
            "xp": xps[ti], "wp": wps[oi], "gw": gw_pack,
            "icsr": icsr.astype(BF),
            "ocs": np.ascontiguousarray(ocs[:, oi * Oc:(oi + 1) * Oc]).astype(BF),
            "bias": bias_cols[oi], "sel": selm.astype(BF),
            "ones41": np.ones((E, 1), dtype=np.float32).astype(BF),
            "ones14": np.ones((1, 100), dtype=np.float32).astype(BF),
        })
    res = run_bass_kernel_spmd(nc, in_maps, core_ids=list(range(NCORES)),
                               trace=TRACE)
    if TRACE:
        LAST_EXEC_NS = res.exec_time_ns
        if res.instructions_and_trace:
            LAST_TRACE_PATH = res.instructions_and_trace[1]
    yfull = np.empty((N, O), dtype=np.float32)
    for c in range(NCORES):
        ti, oi = c // B, c % B
        yfull[ti * Nc:(ti + 1) * Nc, oi * Oc:(oi + 1) * Oc] = \
            res.results[c]["y"].T
    return yfull.reshape(Bsz, S, O)


# revision 33
# speedup vs baseline: 1.2325x; 1.0099x over previous
"""nn_BinaryMoSLinear Trainium2 kernel: 8 NeuronCores, 2 token-halves x 4
out-feature quarters.

kernel(**inputs) takes the FULL reference.setup_inputs() tensors and returns
the FULL [4, 2048, 4096] f32 output. Core c handles token half ti=c//4 and
out-feature quarter oi=c%4 (Nc=4096 tokens, Oc=1024 features per core).

The host pre-binarizes the weight (sign -> bf16, exactly representable) and
casts/transposes x to bf16, so the main matmul runs with a stationary bf16
weight tile [128h x 128o] and a moving activation tile [128h x 512tok] at
1 row/cycle. The token stream is chunked into 8 pairs of 512: while pair k's
256 main matmuls run, pair k+1's router/softmax/in-scale/xs stages are
interleaved into the 8 o-blocks (in-scale matmuls woven into the main
h-loop) so the PE never starves. All K=4 scale matmuls run in bf16 so they
row/col-tile-pack; softmax normalization stays off the PE critical path
(PE broadcasts the exp-sum, DVE does reciprocal+multiply). The xs scaling
multiplies alternate between DVE and the Pool engine. Output is produced
transposed [Oc, Nc] and re-assembled on the host.
"""
from contextlib import ExitStack

import concourse.bass as bass
import concourse.mybir as mybir

F32 = mybir.dt.float32
F32R = mybir.dt.float32r
BF16 = mybir.dt.bfloat16
AF = mybir.ActivationFunctionType
OP = mybir.AluOpType


def f32(ap):
    return ap.bitcast(F32)


def build_kernel(ctx, tc, outs, ins, cfg):
    nc = tc.nc
    H, Oc, Nc, E = cfg["H"], cfg["Oc"], cfg["Nc"], cfg["E"]
    NH = H // 128            # 32 h-chunks
    NO = Oc // 128           # 8 o-tiles
    NP = Nc // 512           # 8 token pairs (tc chunks)
    RING = 3
    y = outs["y"]

    pool = ctx.enter_context(tc.tile_pool(name="sb", bufs=1))

    # ---- persistent SBUF tensors ----
    ring = [pool.tile([128, NH * 512], BF16, name=f"ring{r}", tag=f"ring{r}",
                      bufs=1) for r in range(RING)]
    wq = [pool.tile([128, NH * 256], BF16, name=f"wq{q}", tag=f"wq{q}",
                    bufs=1) for q in range(4)]
    rT_rep = pool.tile([128, Nc], BF16, name="rT_rep", tag="rT", bufs=1)
    ics_rep = pool.tile([128, H], BF16, name="ics_rep", tag="ics", bufs=1)
    gw_pack = pool.tile([128, NH * E], BF16, name="gw_pack", tag="gw", bufs=1)
    ocs_sb = pool.tile([E, Oc], BF16, name="ocs_sb", tag="ocs", bufs=1)
    bias_sb = pool.tile([128, NO], F32, name="bias_sb", tag="bias", bufs=1)
    sel_sb = pool.tile([128, 100], BF16, name="sel_sb", tag="sel", bufs=1)
    ones_e1 = pool.tile([E, 1], BF16, name="ones_e1", tag="o41", bufs=1)
    ones_1e = pool.tile([1, 100], BF16, name="ones_1e", tag="o14", bufs=1)

    # ---- input DMAs: x pairs on the SP hwdge ring, everything else on the
    # ACT ring so the first x pair starts streaming immediately ----
    def x_dma(pr):
        nc.sync.dma_start(ring[pr % RING][:],
                          ins["xp"][pr * 128:(pr + 1) * 128, :])

    # pair 0 in two halves so the router can start on the first half
    hw = NH // 2 * 512
    nc.sync.dma_start(ring[0][:, 0:hw], ins["xp"][0:128, 0:hw])
    nc.sync.dma_start(ring[0][:, hw:], ins["xp"][0:128, hw:])
    for pr in range(1, min(RING, NP)):
        x_dma(pr)
    nc.scalar.dma_start(gw_pack[:], ins["gw"][:, :])
    nc.scalar.dma_start(sel_sb[:], ins["sel"][:, :])
    nc.scalar.dma_start(ones_e1[:], ins["ones41"][:, :])
    nc.scalar.dma_start(ones_1e[:], ins["ones14"][:, :])
    nc.scalar.dma_start(ocs_sb[:], ins["ocs"][:, :])
    nc.scalar.dma_start(bias_sb[:], ins["bias"][:, :])
    nc.scalar.dma_start(ics_rep[:], ins["icsr"][:, :])
    for q in range(4):
        nc.scalar.dma_start(wq[q][:], ins["wp"][q * 128:(q + 1) * 128, :])

    # ---- PSUM pools (2 + 1 + 1 + 4 = 8 banks) ----
    ps_main = ctx.enter_context(
        tc.tile_pool(name="ps_main", bufs=2, space="PSUM"))
    ps_os = ctx.enter_context(tc.tile_pool(name="ps_os", bufs=1, space="PSUM"))
    ps_sm = ctx.enter_context(tc.tile_pool(name="ps_sm", bufs=1, space="PSUM"))
    ps_is = ctx.enter_context(tc.tile_pool(name="ps_is", bufs=4, space="PSUM"))

    class Prep:
        """Stages preparing pair pr: router -> softmax -> in-scale/xs.

        r (softmax weights) is produced in bf16 on partition strips {0-3}
        and {32-35} so the K=4 in-scale matmuls run 2-row-group packed.
        """

        def __init__(self, pr):
            self.pr = pr
            self.rng = ring[pr % RING]
            self.tsl = slice(pr * 512, (pr + 1) * 512)

        def router(self):
            pr = self.pr
            self.strip = ps_sm.tile([128, 512], F32, name=f"strip{pr}",
                                    tag="psm")
            for r in range(NH // 4):
                for c in range(4):
                    h = 4 * r + c
                    nc.tensor.matmul(
                        self.strip[32 * c:32 * c + E, :],
                        gw_pack[:, h * E:(h + 1) * E],
                        self.rng[:, h * 512:(h + 1) * 512],
                        start=(r == 0), stop=(r == NH // 4 - 1),
                        tile_position=(0, 32 * c))
            sc = pool.tile([128, 512], BF16, name=f"sc{pr}", tag="sc", bufs=2)
            nc.vector.tensor_copy(sc[:], self.strip[:])
            self.sc = sc

        def sm_a(self):
            pr = self.pr
            lg = ps_sm.tile([128, 512], F32, name=f"lg{pr}", tag="psm")
            self.ex = pool.tile([128, 512], BF16, name=f"ex{pr}", tag="ex",
                                bufs=2)
            nc.tensor.matmul(lg[0:100, :], sel_sb[:], self.sc[:],
                             start=True, stop=True)
            for c in range(4):
                nc.scalar.activation(self.ex[32 * c:32 * c + E, :],
                                     lg[32 * c:32 * c + E, :], AF.Exp)

        def sm_b(self):
            pr = self.pr
            ssum = ps_sm.tile([1, 512], F32, name=f"ss{pr}", tag="psm")
            nc.tensor.matmul(ssum[:], ones_e1[:], self.ex[0:E, :],
                             start=True, stop=True)
            self.ss = pool.tile([1, 512], BF16, name=f"ssb{pr}", tag="ssb",
                                bufs=2)
            nc.vector.tensor_copy(self.ss[:], ssum[:])

        def sm_c(self):
            pr = self.pr
            bc = ps_sm.tile([128, 512], F32, name=f"bc{pr}", tag="psm")
            nc.tensor.matmul(bc[0:100, :], ones_1e[:], self.ss[:],
                             start=True, stop=True)
            rcp = pool.tile([128, 512], F32R, name=f"rcp{pr}", tag="rcp",
                            bufs=2)
            with nc.allow_low_precision(reason="softmax reciprocal"):
                nc.vector.reciprocal(rcp[0:100, :], bc[0:100, :])
            for c in range(4):
                s = slice(32 * c, 32 * c + E)
                nc.vector.tensor_tensor(rT_rep[s, self.tsl], self.ex[s, :],
                                        f32(rcp[s, :]), OP.mult)

        def is_quad(self, r):
            """4-row-group packed in-scale matmuls for h = 4r..4r+3."""
            pr = self.pr
            isps = []
            for c in range(4):
                h = 4 * r + c
                isp = ps_is.tile([128, 512], F32, name=f"is{pr}_{h}",
                                 tag="psi")
                nc.tensor.matmul(
                    isp[:],
                    ics_rep[32 * c:32 * c + E, h * 128:(h + 1) * 128],
                    rT_rep[32 * c:32 * c + E, self.tsl],
                    start=True, stop=True, tile_position=(32 * c, 0))
                isps.append(isp)
            for c in range(4):
                h = 4 * r + c
                xsl = self.rng[:, h * 512:(h + 1) * 512]
                nc.vector.tensor_tensor(xsl, xsl, isps[c][:], OP.mult)

        # stage placement inside the consuming TC's o-blocks
        def post(self, o):
            if o == 2:
                self.router()
            elif o == 3:
                self.sm_a()
            elif o == 4:
                self.sm_b()

        def insert(self, o, j):
            if o == 5 and j == 3:
                self.sm_c()
            elif o in (6, 7) and j % 8 == 3:
                self.is_quad(4 * (o - 6) + (j - 3) // 8)  # quads 0..7

    # ---- prologue: pair 0 is prepared serially; dummy matmul fill keeps
    # the PE busy (and the HAM clock-gate at 8/8) while the DVE runs the
    # latency-bound softmax/xs chain, so TC0 starts warm and clean ----
    _dummy_seq = [0]

    def dummy_fill(n):
        t = ps_is.tile([E, 128], F32, name=f"dumm{_dummy_seq[0]}", tag="psi")
        _dummy_seq[0] += 1
        for i in range(n):
            nc.tensor.matmul(t[:], gw_pack[:, 0:E], gw_pack[:, 0:128],
                             start=True, stop=True)

    prep0 = Prep(0)
    prep0.router()
    prep0.sm_a()
    dummy_fill(96)
    prep0.sm_b()
    prep0.sm_c()
    dummy_fill(176)
    for r in range(NH // 4):
        prep0.is_quad(r)
        dummy_fill(45)

    # ---- main loop over token pairs ----
    for k in range(NP):
        prep = Prep(k + 1) if k + 1 < NP else None
        rng = ring[k % RING]
        tsl = slice(k * 512, (k + 1) * 512)
        for o in range(NO):
            osp = ps_os.tile([128, 512], F32, name=f"os{k}_{o}", tag="pso")
            nc.tensor.matmul(osp[:], ocs_sb[:, o * 128:(o + 1) * 128],
                             rT_rep[0:E, tsl], start=True, stop=True)
            os_sb = pool.tile([128, 512], F32, name=f"osb{k}_{o}",
                              tag="osb", bufs=2)
            nc.scalar.activation(os_sb[:], osp[:], AF.Copy)
            mp = ps_main.tile([128, 512], F32, name=f"mp{k}_{o}", tag="psm")
            q, half = o // 2, (o % 2) * 128
            for j in range(NH):
                nc.tensor.matmul(
                    mp[:],
                    wq[q][:, j * 256 + half:j * 256 + half + 128],
                    rng[:, j * 512:(j + 1) * 512],
                    start=(j == 0), stop=(j == NH - 1))
                if prep is not None and not (k == 0 and o == 0):
                    prep.insert(o, j)
            if prep is not None:
                prep.post(o)
            y1 = pool.tile([128, 512], F32, name=f"y1_{k}_{o}",
                           tag="y1", bufs=2)
            nc.vector.tensor_tensor(y1[:], mp[:], os_sb[:], OP.mult)
            y2 = pool.tile([128, 512], F32, name=f"y2_{k}_{o}",
                           tag="y2", bufs=3)
            nc.scalar.activation(y2[:], y1[:], AF.Identity,
                                 bias=bias_sb[:, o:o + 1])
            nc.scalar.dma_start(y[o * 128:(o + 1) * 128, tsl], y2[:])
        # refill this ring slot for pair k+RING; emitted after the last
        # reader of the old contents so the overwrite orders correctly
        if k + RING < NP:
            x_dma(k + RING)


import numpy as np
import ml_dtypes

BF = ml_dtypes.bfloat16
NCORES = 8
A, B = 2, 4
Bsz, S, H, O, E = 4, 2048, 4096, 4096, 4
N = Bsz * S
Nc = N // A
Oc = O // B
NH = H // 128
NO = Oc // 128
NP = Nc // 512
CFG = dict(H=H, Oc=Oc, Nc=Nc, E=E)

TRACE = False
LAST_EXEC_NS = None
LAST_TRACE_PATH = None
_NC_CACHE = None


def _get_nc():
    global _NC_CACHE
    if _NC_CACHE is None:
        import concourse.bacc as bacc
        import concourse.tile as tile
        nc = bacc.Bacc("TRN2", target_bir_lowering=False, debug=False,
                       num_devices=NCORES)
        ins_aps = {
            "xp": nc.dram_tensor("xp", [NP * 128, NH * 512], BF16,
                                 kind="ExternalInput").ap(),
            "wp": nc.dram_tensor("wp", [4 * 128, NH * 256], BF16,
                                 kind="ExternalInput").ap(),
            "gw": nc.dram_tensor("gw", [128, NH * E], BF16,
                                 kind="ExternalInput").ap(),
            "icsr": nc.dram_tensor("icsr", [128, H], BF16,
                                   kind="ExternalInput").ap(),
            "ocs": nc.dram_tensor("ocs", [E, Oc], BF16,
                                  kind="ExternalInput").ap(),
            "bias": nc.dram_tensor("bias", [128, NO], F32,
                                   kind="ExternalInput").ap(),
            "sel": nc.dram_tensor("sel", [128, 100], BF16,
                                  kind="ExternalInput").ap(),
            "ones41": nc.dram_tensor("ones41", [E, 1], BF16,
                                     kind="ExternalInput").ap(),
            "ones14": nc.dram_tensor("ones14", [1, 100], BF16,
                                     kind="ExternalInput").ap(),
        }
        outs_aps = {"y": nc.dram_tensor("y", [Oc, Nc], F32,
                                        kind="ExternalOutput").ap()}
        with tile.TileContext(nc) as tc:
            with ExitStack() as ctx:
                build_kernel(ctx, tc, outs_aps, ins_aps, CFG)
        nc.compile()
        _NC_CACHE = nc
    return _NC_CACHE


def kernel(x, weight, bias, gate_w, in_channel_scale, out_channel_scale):
    """Full inputs in, full output out; distributes over 8 NeuronCores."""
    global LAST_EXEC_NS, LAST_TRACE_PATH
    from concourse.bass_utils import run_bass_kernel_spmd

    x = np.asarray(x, dtype=np.float32)
    weight = np.asarray(weight, dtype=np.float32)
    bias = np.asarray(bias, dtype=np.float32)
    gate_w = np.asarray(gate_w, dtype=np.float32)
    ics = np.asarray(in_channel_scale, dtype=np.float32)
    ocs = np.asarray(out_channel_scale, dtype=np.float32)

    nc = _get_nc()
    xf = x.reshape(N, H)
    wsign = np.sign(weight).astype(BF)          # [O, H], exactly +-1

    # x per token-half, swizzled: xp[pr*128+p, j*512+t] = x[half, pr*512+t, j*128+p]
    xps = []
    for ti in range(A):
        xh = xf[ti * Nc:(ti + 1) * Nc, :].astype(BF)         # [Nc, H]
        xp = xh.reshape(NP, 512, NH, 128).transpose(0, 3, 2, 1)
        xps.append(np.ascontiguousarray(xp.reshape(NP * 128, NH * 512)))
    # w per o-quarter-slab, swizzled: wp[q*128+p, j*256+c] = sign(w)[oi*1024+q*256+c, j*128+p]
    wps = []
    for oi in range(B):
        ws = wsign[oi * Oc:(oi + 1) * Oc, :]                 # [1024, H]
        wp = ws.reshape(4, 256, NH, 128).transpose(0, 3, 2, 1)
        wps.append(np.ascontiguousarray(wp.reshape(4 * 128, NH * 256)))

    gw_pack = np.ascontiguousarray(
        gate_w.T.reshape(NH, 128, E).transpose(1, 0, 2).reshape(128, NH * E)
    ).astype(BF)
    icsr = np.zeros((128, H), dtype=np.float32)
    selm = np.zeros((128, 100), dtype=np.float32)
    for c in range(4):
        icsr[32 * c:32 * c + E, :] = ics
        for co in range(4):   # replicate logits onto 4 partition strips
            selm[32 * c + np.arange(E), 32 * co + np.arange(E)] = 1.0
    bias_cols = np.ascontiguousarray(
        bias.reshape(B, NO, 128).transpose(0, 2, 1))         # [B][128, NO]

    in_maps = []
    for c in range(NCORES):
        ti, oi = c // B, c % B
        in_maps.append({
            "xp": xps[ti], "wp": wps[oi], "gw": gw_pack,
            "icsr": icsr.astype(BF),
            "ocs": np.ascontiguousarray(ocs[:, oi * Oc:(oi + 1) * Oc]).astype(BF),
            "bias": bias_cols[oi], "sel": selm.astype(BF),
            "ones41": np.ones((E, 1), dtype=np.float32).astype(BF),
            "ones14": np.ones((1, 100), dtype=np.float32).astype(BF),
        })
    res = run_bass_kernel_spmd(nc, in_maps, core_ids=list(range(NCORES)),
                               trace=TRACE)
    if TRACE:
        LAST_EXEC_NS = res.exec_time_ns
        if res.instructions_and_trace:
            LAST_TRACE_PATH = res.instructions_and_trace[1]
    yfull = np.empty((N, O), dtype=np.float32)
    for c in range(NCORES):
        ti, oi = c // B, c % B
        yfull[ti * Nc:(ti + 1) * Nc, oi * Oc:(oi + 1) * Oc] = \
            res.results[c]["y"].T
    return yfull.reshape(Bsz, S, O)


# revision 35
# speedup vs baseline: 1.2793x; 1.0379x over previous
"""nn_BinaryMoSLinear Trainium2 kernel: 8 NeuronCores, 2 token-halves x 4
out-feature quarters.

kernel(**inputs) takes the FULL reference.setup_inputs() tensors and returns
the FULL [4, 2048, 4096] f32 output. Core c handles token half ti=c//4 and
out-feature quarter oi=c%4 (Nc=4096 tokens, Oc=1024 features per core).

The host pre-binarizes the weight (sign -> bf16, exactly representable) and
casts/transposes x to bf16, so the main matmul runs with a stationary bf16
weight tile [128h x 128o] and a moving activation tile [128h x 512tok] at
1 row/cycle. The token stream is chunked into 8 pairs of 512: while pair k's
256 main matmuls run, pair k+1's router/softmax/in-scale/xs stages are
interleaved into the 8 o-blocks (in-scale matmuls woven into the main
h-loop) so the PE never starves. All K=4 scale matmuls run in bf16 so they
row/col-tile-pack; softmax normalization stays off the PE critical path
(PE broadcasts the exp-sum, DVE does reciprocal+multiply). The xs scaling
multiplies alternate between DVE and the Pool engine. Output is produced
transposed [Oc, Nc] and re-assembled on the host.
"""
from contextlib import ExitStack

import concourse.bass as bass
import concourse.mybir as mybir

F32 = mybir.dt.float32
F32R = mybir.dt.float32r
BF16 = mybir.dt.bfloat16
AF = mybir.ActivationFunctionType
OP = mybir.AluOpType


def f32(ap):
    return ap.bitcast(F32)


def build_kernel(ctx, tc, outs, ins, cfg):
    nc = tc.nc
    H, Oc, Nc, E = cfg["H"], cfg["Oc"], cfg["Nc"], cfg["E"]
    NH = H // 128            # 32 h-chunks
    NO = Oc // 128           # 8 o-tiles
    NP = Nc // 512           # 8 token pairs (tc chunks)
    RING = 3
    y = outs["y"]

    pool = ctx.enter_context(tc.tile_pool(name="sb", bufs=1))

    # ---- persistent SBUF tensors ----
    ring = [pool.tile([128, NH * 512], BF16, name=f"ring{r}", tag=f"ring{r}",
                      bufs=1) for r in range(RING)]
    wq = [pool.tile([128, NH * 256], BF16, name=f"wq{q}", tag=f"wq{q}",
                    bufs=1) for q in range(4)]
    rT_rep = pool.tile([128, Nc], BF16, name="rT_rep", tag="rT", bufs=1)
    ics_rep = pool.tile([128, H], BF16, name="ics_rep", tag="ics", bufs=1)
    gw_pack = pool.tile([128, NH * E], BF16, name="gw_pack", tag="gw", bufs=1)
    ocs_sb = pool.tile([E, Oc], BF16, name="ocs_sb", tag="ocs", bufs=1)
    bias_sb = pool.tile([128, NO], F32, name="bias_sb", tag="bias", bufs=1)
    sel_sb = pool.tile([128, 100], BF16, name="sel_sb", tag="sel", bufs=1)
    ones_e1 = pool.tile([E, 1], BF16, name="ones_e1", tag="o41", bufs=1)
    ones_1e = pool.tile([1, 100], BF16, name="ones_1e", tag="o14", bufs=1)

    # ---- input DMAs: x pairs on the SP hwdge ring, everything else on the
    # ACT ring so the first x pair starts streaming immediately ----
    def x_dma(pr):
        nc.sync.dma_start(ring[pr % RING][:],
                          ins["xp"][pr * 128:(pr + 1) * 128, :])

    # pair 0 in quarters so the router can start on the first chunk
    qw = NH // 4 * 512
    for qq in range(4):
        nc.sync.dma_start(ring[0][:, qq * qw:(qq + 1) * qw],
                          ins["xp"][0:128, qq * qw:(qq + 1) * qw])
    for pr in range(1, min(RING, NP)):
        x_dma(pr)
    nc.scalar.dma_start(gw_pack[:], ins["gw"][:, :])
    nc.scalar.dma_start(sel_sb[:], ins["sel"][:, :])
    nc.scalar.dma_start(ones_e1[:], ins["ones41"][:, :])
    nc.scalar.dma_start(ones_1e[:], ins["ones14"][:, :])
    nc.scalar.dma_start(ocs_sb[:], ins["ocs"][:, :])
    nc.scalar.dma_start(bias_sb[:], ins["bias"][:, :])
    nc.scalar.dma_start(ics_rep[:], ins["icsr"][:, :])
    for q in range(4):
        nc.scalar.dma_start(wq[q][:], ins["wp"][q * 128:(q + 1) * 128, :])

    # ---- PSUM pools (2 + 1 + 1 + 4 = 8 banks) ----
    ps_main = ctx.enter_context(
        tc.tile_pool(name="ps_main", bufs=2, space="PSUM"))
    ps_os = ctx.enter_context(tc.tile_pool(name="ps_os", bufs=1, space="PSUM"))
    ps_sm = ctx.enter_context(tc.tile_pool(name="ps_sm", bufs=1, space="PSUM"))
    ps_is = ctx.enter_context(tc.tile_pool(name="ps_is", bufs=4, space="PSUM"))

    class Prep:
        """Stages preparing pair pr: router -> softmax -> in-scale/xs.

        r (softmax weights) is produced in bf16 on partition strips {0-3}
        and {32-35} so the K=4 in-scale matmuls run 2-row-group packed.
        """

        def __init__(self, pr):
            self.pr = pr
            self.rng = ring[pr % RING]
            self.tsl = slice(pr * 512, (pr + 1) * 512)

        def router(self):
            pr = self.pr
            self.strip = ps_sm.tile([128, 512], F32, name=f"strip{pr}",
                                    tag="psm")
            for r in range(NH // 4):
                for c in range(4):
                    h = 4 * r + c
                    nc.tensor.matmul(
                        self.strip[32 * c:32 * c + E, :],
                        gw_pack[:, h * E:(h + 1) * E],
                        self.rng[:, h * 512:(h + 1) * 512],
                        start=(r == 0), stop=(r == NH // 4 - 1),
                        tile_position=(0, 32 * c))
            sc = pool.tile([128, 512], BF16, name=f"sc{pr}", tag="sc", bufs=2)
            nc.vector.tensor_copy(sc[:], self.strip[:])
            self.sc = sc

        def sm_a(self):
            pr = self.pr
            lg = ps_sm.tile([128, 512], F32, name=f"lg{pr}", tag="psm")
            self.ex = pool.tile([128, 512], BF16, name=f"ex{pr}", tag="ex",
                                bufs=2)
            nc.tensor.matmul(lg[0:100, :], sel_sb[:], self.sc[:],
                             start=True, stop=True)
            for c in range(4):
                nc.scalar.activation(self.ex[32 * c:32 * c + E, :],
                                     lg[32 * c:32 * c + E, :], AF.Exp)

        def sm_b(self):
            pr = self.pr
            ssum = ps_sm.tile([1, 512], F32, name=f"ss{pr}", tag="psm")
            nc.tensor.matmul(ssum[:], ones_e1[:], self.ex[0:E, :],
                             start=True, stop=True)
            self.ss = pool.tile([1, 512], BF16, name=f"ssb{pr}", tag="ssb",
                                bufs=2)
            nc.vector.tensor_copy(self.ss[:], ssum[:])

        def sm_c(self):
            pr = self.pr
            bc = ps_sm.tile([128, 512], F32, name=f"bc{pr}", tag="psm")
            nc.tensor.matmul(bc[0:100, :], ones_1e[:], self.ss[:],
                             start=True, stop=True)
            rcp = pool.tile([128, 512], F32R, name=f"rcp{pr}", tag="rcp",
                            bufs=2)
            with nc.allow_low_precision(reason="softmax reciprocal"):
                nc.vector.reciprocal(rcp[0:100, :], bc[0:100, :])
            for c in range(4):
                s = slice(32 * c, 32 * c + E)
                nc.vector.tensor_tensor(rT_rep[s, self.tsl], self.ex[s, :],
                                        f32(rcp[s, :]), OP.mult)

        def is_quad(self, r):
            """4-row-group packed in-scale matmuls for h = 4r..4r+3.

            The xs multiplies alternate between the DVE (straight from
            PSUM) and ACT-copy + Pool-engine multiply, splitting the
            elementwise load across three otherwise-idle engines.
            """
            pr = self.pr
            isps = []
            for c in range(4):
                h = 4 * r + c
                isp = ps_is.tile([128, 512], F32, name=f"is{pr}_{h}",
                                 tag="psi")
                nc.tensor.matmul(
                    isp[:],
                    ics_rep[32 * c:32 * c + E, h * 128:(h + 1) * 128],
                    rT_rep[32 * c:32 * c + E, self.tsl],
                    start=True, stop=True, tile_position=(32 * c, 0))
                isps.append(isp)
            for c in range(4):
                h = 4 * r + c
                xsl = self.rng[:, h * 512:(h + 1) * 512]
                if c % 2 == 0:
                    nc.vector.tensor_tensor(xsl, xsl, isps[c][:], OP.mult)
                else:
                    is_sb = pool.tile([128, 512], F32, name=f"isb{pr}_{h}",
                                      tag="isb", bufs=2)
                    nc.scalar.activation(is_sb[:], isps[c][:], AF.Copy)
                    nc.gpsimd.tensor_tensor(xsl, xsl, is_sb[:], OP.mult)

        # stage placement inside the consuming TC's o-blocks
        def post(self, o):
            if o == 2:
                self.router()
            elif o == 3:
                self.sm_a()
            elif o == 4:
                self.sm_b()

        def insert(self, o, j):
            if o == 5 and j == 3:
                self.sm_c()
            elif o in (6, 7) and j % 8 == 3:
                self.is_quad(4 * (o - 6) + (j - 3) // 8)  # quads 0..7

    # ---- prologue: pair 0 is prepared serially; dummy matmul fill keeps
    # the PE busy (and the HAM clock-gate at 8/8) while the DVE runs the
    # latency-bound softmax/xs chain, so TC0 starts warm and clean ----
    _dummy_seq = [0]

    def dummy_fill(n):
        t = ps_is.tile([E, 128], F32, name=f"dumm{_dummy_seq[0]}", tag="psi")
        _dummy_seq[0] += 1
        for i in range(n):
            nc.tensor.matmul(t[:], gw_pack[:, 0:E], gw_pack[:, 0:128],
                             start=True, stop=True)

    prep0 = Prep(0)
    prep0.router()
    prep0.sm_a()
    dummy_fill(96)
    prep0.sm_b()
    prep0.sm_c()
    dummy_fill(176)
    for r in range(NH // 4):
        prep0.is_quad(r)
        dummy_fill(45)

    # ---- main loop over token pairs ----
    for k in range(NP):
        prep = Prep(k + 1) if k + 1 < NP else None
        rng = ring[k % RING]
        tsl = slice(k * 512, (k + 1) * 512)
        for o in range(NO):
            osp = ps_os.tile([128, 512], F32, name=f"os{k}_{o}", tag="pso")
            nc.tensor.matmul(osp[:], ocs_sb[:, o * 128:(o + 1) * 128],
                             rT_rep[0:E, tsl], start=True, stop=True)
            os_sb = pool.tile([128, 512], F32, name=f"osb{k}_{o}",
                              tag="osb", bufs=2)
            nc.scalar.activation(os_sb[:], osp[:], AF.Copy)
            mp = ps_main.tile([128, 512], F32, name=f"mp{k}_{o}", tag="psm")
            q, half = o // 2, (o % 2) * 128
            for j in range(NH):
                nc.tensor.matmul(
                    mp[:],
                    wq[q][:, j * 256 + half:j * 256 + half + 128],
                    rng[:, j * 512:(j + 1) * 512],
                    start=(j == 0), stop=(j == NH - 1))
                if prep is not None and not (k == 0 and o == 0):
                    prep.insert(o, j)
            if prep is not None:
                prep.post(o)
            y1 = pool.tile([128, 512], F32, name=f"y1_{k}_{o}",
                           tag="y1", bufs=2)
            nc.vector.tensor_tensor(y1[:], mp[:], os_sb[:], OP.mult)
            y2 = pool.tile([128, 512], F32, name=f"y2_{k}_{o}",
                           tag="y2", bufs=3)
            nc.scalar.activation(y2[:], y1[:], AF.Identity,
                                 bias=bias_sb[:, o:o + 1])
            nc.scalar.dma_start(y[o * 128:(o + 1) * 128, tsl], y2[:])
        # refill this ring slot for pair k+RING; emitted after the last
        # reader of the old contents so the overwrite orders correctly
        if k + RING < NP:
            x_dma(k + RING)


import numpy as np
import ml_dtypes

BF = ml_dtypes.bfloat16
NCORES = 8
A, B = 2, 4
Bsz, S, H, O, E = 4, 2048, 4096, 4096, 4
N = Bsz * S
Nc = N // A
Oc = O // B
NH = H // 128
NO = Oc // 128
NP = Nc // 512
CFG = dict(H=H, Oc=Oc, Nc=Nc, E=E)

TRACE = False
LAST_EXEC_NS = None
LAST_TRACE_PATH = None
_NC_CACHE = None


def _get_nc():
    global _NC_CACHE
    if _NC_CACHE is None:
        import concourse.bacc as bacc
        import concourse.tile as tile
        nc = bacc.Bacc("TRN2", target_bir_lowering=False, debug=False,
                       num_devices=NCORES)
        ins_aps = {
            "xp": nc.dram_tensor("xp", [NP * 128, NH * 512], BF16,
                                 kind="ExternalInput").ap(),
            "wp": nc.dram_tensor("wp", [4 * 128, NH * 256], BF16,
                                 kind="ExternalInput").ap(),
            "gw": nc.dram_tensor("gw", [128, NH * E], BF16,
                                 kind="ExternalInput").ap(),
            "icsr": nc.dram_tensor("icsr", [128, H], BF16,
                                   kind="ExternalInput").ap(),
            "ocs": nc.dram_tensor("ocs", [E, Oc], BF16,
                                  kind="ExternalInput").ap(),
            "bias": nc.dram_tensor("bias", [128, NO], F32,
                                   kind="ExternalInput").ap(),
            "sel": nc.dram_tensor("sel", [128, 100], BF16,
                                  kind="ExternalInput").ap(),
            "ones41": nc.dram_tensor("ones41", [E, 1], BF16,
                                     kind="ExternalInput").ap(),
            "ones14": nc.dram_tensor("ones14", [1, 100], BF16,
                                     kind="ExternalInput").ap(),
        }
        outs_aps = {"y": nc.dram_tensor("y", [Oc, Nc], F32,
                                        kind="ExternalOutput").ap()}
        with tile.TileContext(nc) as tc:
            with ExitStack() as ctx:
                build_kernel(ctx, tc, outs_aps, ins_aps, CFG)
        nc.compile()
        _NC_CACHE = nc
    return _NC_CACHE


def kernel(x, weight, bias, gate_w, in_channel_scale, out_channel_scale):
    """Full inputs in, full output out; distributes over 8 NeuronCores."""
    global LAST_EXEC_NS, LAST_TRACE_PATH
    from concourse.bass_utils import run_bass_kernel_spmd

    x = np.asarray(x, dtype=np.float32)
    weight = np.asarray(weight, dtype=np.float32)
    bias = np.asarray(bias, dtype=np.float32)
    gate_w = np.asarray(gate_w, dtype=np.float32)
    ics = np.asarray(in_channel_scale, dtype=np.float32)
    ocs = np.asarray(out_channel_scale, dtype=np.float32)

    nc = _get_nc()
    xf = x.reshape(N, H)
    wsign = np.sign(weight).astype(BF)          # [O, H], exactly +-1

    # x per token-half, swizzled: xp[pr*128+p, j*512+t] = x[half, pr*512+t, j*128+p]
    xps = []
    for ti in range(A):
        xh = xf[ti * Nc:(ti + 1) * Nc, :].astype(BF)         # [Nc, H]
        xp = xh.reshape(NP, 512, NH, 128).transpose(0, 3, 2, 1)
        xps.append(np.ascontiguousarray(xp.reshape(NP * 128, NH * 512)))
    # w per o-quarter-slab, swizzled: wp[q*128+p, j*256+c] = sign(w)[oi*1024+q*256+c, j*128+p]
    wps = []
    for oi in range(B):
        ws = wsign[oi * Oc:(oi + 1) * Oc, :]                 # [1024, H]
        wp = ws.reshape(4, 256, NH, 128).transpose(0, 3, 2, 1)
        wps.append(np.ascontiguousarray(wp.reshape(4 * 128, NH * 256)))

    gw_pack = np.ascontiguousarray(
        gate_w.T.reshape(NH, 128, E).transpose(1, 0, 2).reshape(128, NH * E)
    ).astype(BF)
    icsr = np.zeros((128, H), dtype=np.float32)
    selm = np.zeros((128, 100), dtype=np.float32)
    for c in range(4):
        icsr[32 * c:32 * c + E, :] = ics
        for co in range(4):   # replicate logits onto 4 partition strips
            selm[32 * c + np.arange(E), 32 * co + np.arange(E)] = 1.0
    bias_cols = np.ascontiguousarray(
        bias.reshape(B, NO, 128).transpose(0, 2, 1))         # [B][128, NO]

    in_maps = []
    for c in range(NCORES):
        ti, oi = c // B, c % B
        in_maps.append({
            "xp": xps[ti], "wp": wps[oi], "gw": gw_pack,
            "icsr": icsr.astype(BF),
            "ocs": np.ascontiguousarray(ocs[:, oi * Oc:(oi + 1) * Oc]).astype(BF),
            "bias": bias_cols[oi], "sel": selm.astype(BF),
            "ones41": np.ones((E, 1), dtype=np.float32).astype(BF),
            "ones14": np.ones((1, 100), dtype=np.float32).astype(BF),
        })
    res = run_bass_kernel_spmd(nc, in_maps, core_ids=list(range(NCORES)),
                               trace=TRACE)
    if TRACE:
        LAST_EXEC_NS = res.exec_time_ns
        if res.instructions_and_trace:
            LAST_TRACE_PATH = res.instructions_and_trace[1]
    yfull = np.empty((N, O), dtype=np.float32)
    for c in range(NCORES):
        ti, oi = c // B, c % B
        yfull[ti * Nc:(ti + 1) * Nc, oi * Oc:(oi + 1) * Oc] = \
            res.results[c]["y"].T
    return yfull.reshape(Bsz, S, O)
